# revision 1
# baseline (speedup 1.0000x reference)
"""Multi-head attention (16 heads, S=2048, E=1024, D=M=64, O=1024) on 8 trn2
NeuronCores, head-sharded: 2 heads per core, partial output summed on host.

Self-contained: hardcodes all shapes; builds a Bass program and runs it via
concourse.bass_utils.run_bass_kernel_spmd on cores 0-7.
"""

import os
import sys

import numpy as np

# hardcoded problem shapes
H, E, D, MD, O, S = 16, 1024, 64, 64, 1024, 2048
NCORES = 8
HPC = H // NCORES          # heads per core = 2
DD = HPC * D               # packed head dim rows = 128
P = 128

# filled by the last device run (for test harness)
LAST_EXEC_TIME_NS = None
LAST_RESULTS = None

_REPO = "/opt/trn_rl_repo"
if _REPO not in sys.path:
    sys.path.insert(0, _REPO)

_built = {}


def _build_bass():
    import concourse.bass as bass
    import concourse.mybir as mybir
    from concourse.masks import make_identity

    F32 = mybir.dt.float32
    F32R = mybir.dt.float32r
    Exp = mybir.ActivationFunctionType.Exp

    nc = bass.Bass()
    import contextlib
    _lp = contextlib.ExitStack()
    _lp.enter_context(nc.allow_low_precision(
        reason="f32r storage is bit-identical to f32; rounding is intended"))

    xT = nc.declare_dram_parameter("xT", [E, S], F32R, isOutput=False)
    zT = nc.declare_dram_parameter("zT", [E, S], F32R, isOutput=False)
    wq = nc.declare_dram_parameter("wq", [E, DD], F32R, isOutput=False)
    wk = nc.declare_dram_parameter("wk", [E, DD], F32R, isOutput=False)
    wv = nc.declare_dram_parameter("wv", [E, DD], F32R, isOutput=False)
    bq = nc.declare_dram_parameter("bq", [DD, 1], F32, isOutput=False)
    bk = nc.declare_dram_parameter("bk", [DD, 1], F32, isOutput=False)
    bv = nc.declare_dram_parameter("bv", [DD, 1], F32, isOutput=False)
    w0 = nc.declare_dram_parameter("w0", [DD, O], F32R, isOutput=False)
    out = nc.declare_dram_parameter("out", [S, O], F32, isOutput=True)

    EC = E // P               # 8 e-chunks
    SC = S // 512             # 4 s-chunks of 512
    TB = S // P               # 16 t-blocks
    NEX = 3                   # exp sbuf slots
    NOB = 4                   # output staging slots

    # ---- static SBUF allocation --------------------------------------
    xt_sb = nc.alloc_sbuf_tensor("xt_sb", [P, EC, S], F32R).ap()
    zt_sb = nc.alloc_sbuf_tensor("zt_sb", [P, EC, S], F32R).ap()
    qT_sb = nc.alloc_sbuf_tensor("qT_sb", [P, S], F32R).ap()
    kT_sb = nc.alloc_sbuf_tensor("kT_sb", [P, S], F32R).ap()
    vT_sb = nc.alloc_sbuf_tensor("vT_sb", [P, S], F32R).ap()
    wq_sb = nc.alloc_sbuf_tensor("wq_sb", [P, EC, DD], F32R).ap()
    wk_sb = nc.alloc_sbuf_tensor("wk_sb", [P, EC, DD], F32R).ap()
    wv_sb = nc.alloc_sbuf_tensor("wv_sb", [P, EC, DD], F32R).ap()
    w0_sb = nc.alloc_sbuf_tensor("w0_sb", [P, O], F32R).ap()
    bq_sb = nc.alloc_sbuf_tensor("bq_sb", [P, 1], F32).ap()
    bk_sb = nc.alloc_sbuf_tensor("bk_sb", [P, 1], F32).ap()
    bv_sb = nc.alloc_sbuf_tensor("bv_sb", [P, 1], F32).ap()
    ones_row = nc.alloc_sbuf_tensor("ones_row", [1, 64], F32R).ap()
    ident = nc.alloc_sbuf_tensor("ident", [P, P], F32R).ap()
    v0_sb = nc.alloc_sbuf_tensor("v0_sb", [P, TB, 65], F32R).ap()
    v1_sb = nc.alloc_sbuf_tensor("v1_sb", [P, TB, 65], F32R).ap()
    ex_sb = nc.alloc_sbuf_tensor("ex_sb", [P, NEX, 1024], F32R).ap()
    rr_sb = nc.alloc_sbuf_tensor("rr_sb", [1, 2, 512], F32R).ap()
    bcs_sb = nc.alloc_sbuf_tensor("bcs_sb", [64, 512], F32).ap()
    oT_sb = nc.alloc_sbuf_tensor("oT_sb", [P, 2, 512], F32R).ap()
    ob_sb = nc.alloc_sbuf_tensor("ob_sb", [P, NOB, 512], F32).ap()

    # ---- static PSUM banks -------------------------------------------
    qa0 = nc.alloc_psum_tensor("qa0", [P, 1024], F32).ap()   # banks 0-1
    qa1 = nc.alloc_psum_tensor("qa1", [P, 1024], F32).ap()   # banks 2-3
    av0 = nc.alloc_psum_tensor("av0", [P, 512], F32).ap()    # bank 4
    av1 = nc.alloc_psum_tensor("av1", [P, 512], F32).ap()    # bank 5
    bcp = nc.alloc_psum_tensor("bcp", [P, 512], F32).ap()    # bank 6
    pjp = nc.alloc_psum_tensor("pjp", [P, 512], F32).ap()    # bank 7

    # ---- semaphores ---------------------------------------------------
    sWTS = nc.alloc_semaphore("sWTS")                       # 6 small tensors
    sW0 = nc.alloc_semaphore("sW0")
    sZT = [nc.alloc_semaphore(f"sZT{c}") for c in range(EC)]
    sXT = [nc.alloc_semaphore(f"sXT{c}") for c in range(EC)]
    sOB = [nc.alloc_semaphore(f"sOB{j}") for j in range(NOB)]
    sPE = nc.alloc_semaphore("sPE")
    sACT = nc.alloc_semaphore("sACT")
    sDVE = nc.alloc_semaphore("sDVE")
    sGP = nc.alloc_semaphore("sGP")

    # ---- closed-form tick schedules ----------------------------------
    # PE sem increments, in order: q groups (4), k (4), v (4),
    # transposes (16), then per sc: [scores, avpair] x 16, bcast x 2,
    # proj x 8  -> 42 per sc.
    def pe_qkv(which, sc):
        return {"q": 0, "k": 4, "v": 8}[which] + sc + 1

    def pe_tp(tb):
        return 12 + tb + 1

    def pe_scores(sc, tb):
        return PE_TICK[("scores", sc, tb)]

    def pe_avpair(sc, tb):
        return PE_TICK[("avpair", sc, tb)]

    def pe_bcast(sc, h):
        return PE_TICK[("bcast", sc, h)]

    def pe_proj(sc, sb, oc):
        return PE_TICK[("proj", sc, sb, oc)]

    # ACT sem: one per exp
    def act_exp(sc, tb):
        return sc * TB + tb + 1

    # DVE sem increments, in order: bias q(4) k(4) v(4), vcopy(16),
    # then per sc: recip x2, [bcs, mult] x2, ob x8 -> 14 per sc.
    def dve_bias(which, sc):
        return {"q": 0, "k": 4, "v": 8}[which] + sc + 1

    def dve_vcopy(tb):
        return 12 + tb + 1

    def dve_rowcp(sc, h):
        return 28 + sc * 14 + h + 1

    def dve_bcs(sc, h):
        return 28 + sc * 14 + 2 + 2 * h + 1

    def dve_div(sc, h):
        return 28 + sc * 14 + 2 + 2 * h + 2

    def dve_ob(gi):
        sc, j = divmod(gi, 8)
        return 28 + sc * 14 + 6 + j + 1

    # software-pipelined PE attention order: scores run 2 iterations
    # ahead of AV; next-chunk scores are issued before the norm/proj of
    # the current chunk so ACT never starves.
    ATTN_ORD = [("scores", 0, 0), ("scores", 0, 1)]
    for sc_ in range(SC):
        for tb_ in range(TB):
            ATTN_ORD.append(("avpair", sc_, tb_))
            gn = sc_ * TB + tb_ + 2
            if gn < SC * TB:
                ATTN_ORD.append(("scores", gn // TB, gn % TB))
            if tb_ == TB - 1:
                for h_ in range(2):
                    ATTN_ORD.append(("bcast", sc_, h_))
                for sb_ in range(4):
                    for oc_ in range(2):
                        ATTN_ORD.append(("proj", sc_, sb_, oc_))
    PE_TICK = {e: 28 + i + 1 for i, e in enumerate(ATTN_ORD)}

    counts = {"PE": 0, "ACT": 0, "DVE": 0}

    def inc(eng, instr, sem, expect):
        instr.then_inc(sem, 1)
        counts[eng] += 1
        assert counts[eng] == expect, (eng, counts[eng], expect)

    class WaitTracker:
        def __init__(self, eng):
            self.eng = eng
            self.seen = {}

        def need(self, sem, val):
            if val <= 0:
                return
            key = sem.name
            if self.seen.get(key, -1) >= val:
                return
            self.seen[key] = val
            self.eng.wait_ge(sem, val)

    with nc.Block() as block:

        @block.sync
        def _(sp):
            sp.dma_start(out=wq_sb, in_=wq.rearrange("(c p) d -> p c d", p=P)).then_inc(sWTS, 16)
            sp.dma_start(out=wk_sb, in_=wk.rearrange("(c p) d -> p c d", p=P)).then_inc(sWTS, 16)
            sp.dma_start(out=wv_sb, in_=wv.rearrange("(c p) d -> p c d", p=P)).then_inc(sWTS, 16)
            sp.dma_start(out=bq_sb, in_=bq[:, :]).then_inc(sWTS, 16)
            sp.dma_start(out=bk_sb, in_=bk[:, :]).then_inc(sWTS, 16)
            sp.dma_start(out=bv_sb, in_=bv[:, :]).then_inc(sWTS, 16)
            for c in range(EC):
                sp.dma_start(out=xt_sb[:, c, :], in_=xT[c * P:(c + 1) * P, :]).then_inc(sXT[c], 16)
            for c in range(EC):
                sp.dma_start(out=zt_sb[:, c, :], in_=zT[c * P:(c + 1) * P, :]).then_inc(sZT[c], 16)
            sp.dma_start(out=w0_sb, in_=w0[:, :]).then_inc(sW0, 16)
            w = WaitTracker(sp)
            for sc in range(SC):
                for sb in range(4):
                    row = sc * 512 + sb * P
                    for oc in range(2):
                        gi = sc * 8 + sb * 2 + oc
                        w.need(sDVE, dve_ob(gi))
                        sp.dma_start(
                            out=out[row:row + P, oc * 512:(oc + 1) * 512],
                            in_=ob_sb[:, gi % NOB, :],
                        ).then_inc(sOB[gi % NOB], 16)
            for j in range(NOB):
                sp.wait_ge(sOB[j], 16 * (SC * 8 // NOB))

        @block.gpsimd
        def _(gp):
            gp.wait_ge(sGP, 1)
            make_identity(nc, ident, nomemset=True)
            nc.gpsimd.engine_nop().then_inc(sGP, 1)

        @block.tensor
        def _(pe):
            w = WaitTracker(pe)
            for ec in range(EC):
                w.need(sWTS, 96)
                w.need(sXT[ec], 16)
                for sc in range(SC):
                    i = nc.tensor.matmul(
                        (qa0 if sc < 2 else qa1)[:, (sc % 2) * 512:(sc % 2) * 512 + 512],
                        lhsT=wq_sb[:, ec, :],
                        rhs=xt_sb[:, ec, sc * 512:(sc + 1) * 512],
                        start=(ec == 0), stop=(ec == EC - 1),
                        skip_group_check=True,
                    )
                    if ec == EC - 1:
                        inc("PE", i, sPE, pe_qkv("q", sc))
            for which, w_sb in (("k", wk_sb), ("v", wv_sb)):
                for ec in range(EC):
                    w.need(sZT[ec], 16)
                    for sc in range(SC):
                        if ec == 0:
                            prev = {"k": "q", "v": "k"}[which]
                            w.need(sDVE, dve_bias(prev, sc))
                        i = nc.tensor.matmul(
                            (qa0 if sc < 2 else qa1)[:, (sc % 2) * 512:(sc % 2) * 512 + 512],
                            lhsT=w_sb[:, ec, :],
                            rhs=zt_sb[:, ec, sc * 512:(sc + 1) * 512],
                            start=(ec == 0), stop=(ec == EC - 1),
                            skip_group_check=True,
                        )
                        if ec == EC - 1:
                            inc("PE", i, sPE, pe_qkv(which, sc))
            w.need(sGP, 2)
            for tb in range(TB):
                tgt = (bcp if tb % 2 == 0 else pjp)[0:P, 0:P].bitcast(F32R)
                w.need(sDVE, dve_bias("v", tb // 4))
                if tb >= 2:
                    w.need(sDVE, dve_vcopy(tb - 2))
                i = nc.tensor.transpose(tgt, vT_sb[:, tb * P:(tb + 1) * P], ident)
                inc("PE", i, sPE, pe_tp(tb))
            w.need(sDVE, dve_vcopy(TB - 1))
            for ent in ATTN_ORD:
                kind = ent[0]
                if kind == "scores":
                    _, sc, tb = ent
                    s0 = sc * 512
                    g = sc * TB + tb
                    qa = qa0 if tb % 2 == 0 else qa1
                    if g >= 2:
                        w.need(sACT, g - 1)
                    nc.tensor.matmul(
                        qa[:, 0:512],
                        lhsT=kT_sb[0:64, tb * P:(tb + 1) * P],
                        rhs=qT_sb[0:64, s0:s0 + 512],
                        start=True, stop=True,
                        tile_position=(0, 0),
                    )
                    i = nc.tensor.matmul(
                        qa[:, 512:1024],
                        lhsT=kT_sb[64:128, tb * P:(tb + 1) * P],
                        rhs=qT_sb[64:128, s0:s0 + 512],
                        start=True, stop=True,
                        tile_position=(64, 0),
                    )
                    inc("PE", i, sPE, pe_scores(sc, tb))
                elif kind == "avpair":
                    _, sc, tb = ent
                    g = sc * TB + tb
                    if tb == 0 and sc > 0:
                        w.need(sDVE, dve_div(sc - 1, 1))
                    w.need(sACT, act_exp(sc, tb))
                    slot = g % NEX
                    nc.tensor.matmul(
                        av0[0:65, :],
                        lhsT=v0_sb[:, tb, :],
                        rhs=ex_sb[:, slot, 0:512],
                        start=(tb == 0), stop=(tb == TB - 1),
                        skip_group_check=True,
                    )
                    i = nc.tensor.matmul(
                        av1[0:65, :],
                        lhsT=v1_sb[:, tb, :],
                        rhs=ex_sb[:, slot, 512:1024],
                        start=(tb == 0), stop=(tb == TB - 1),
                        skip_group_check=True,
                    )
                    inc("PE", i, sPE, pe_avpair(sc, tb))
                elif kind == "bcast":
                    _, sc, h = ent
                    w.need(sDVE, dve_rowcp(sc, h))
                    if h == 1:
                        w.need(sDVE, dve_bcs(sc, 0))
                    elif sc > 0:
                        w.need(sDVE, dve_bcs(sc - 1, 1))
                    i = nc.tensor.matmul(
                        bcp[0:64, :],
                        lhsT=ones_row[0:1, :],
                        rhs=rr_sb[0:1, h, :],
                        start=True, stop=True,
                    )
                    inc("PE", i, sPE, pe_bcast(sc, h))
                else:
                    _, sc, sb, oc = ent
                    gi = sc * 8 + sb * 2 + oc
                    bank = pjp if gi % 2 == 0 else bcp
                    w.need(sW0, 16)
                    w.need(sDVE, dve_div(sc, 1))
                    if gi >= 2:
                        w.need(sDVE, dve_ob(gi - 2))
                    i = nc.tensor.matmul(
                        bank[:, :],
                        lhsT=oT_sb[:, sc % 2, sb * P:(sb + 1) * P],
                        rhs=w0_sb[:, oc * 512:(oc + 1) * 512],
                        start=True, stop=True,
                    )
                    inc("PE", i, sPE, pe_proj(sc, sb, oc))

        @block.scalar
        def _(act):
            w = WaitTracker(act)
            for sc in range(SC):
                for tb in range(TB):
                    gexp = sc * TB + tb
                    w.need(sPE, pe_scores(sc, tb))
                    if gexp >= NEX:
                        gp_sc, gp_tb = divmod(gexp - NEX, TB)
                        w.need(sPE, pe_avpair(gp_sc, gp_tb))
                    slot = gexp % NEX
                    qa = qa0 if tb % 2 == 0 else qa1
                    i = nc.scalar.activation(
                        ex_sb[:, slot, :], qa[:, :], Exp, scale=0.125)
                    inc("ACT", i, sACT, act_exp(sc, tb))

        @block.vector
        def _(dve):
            w = WaitTracker(dve)
            nc.vector.memset(ident.bitcast(F32), 0.0).then_inc(sGP, 1)
            nc.vector.memset(ones_row.bitcast(F32), 1.0)
            nc.vector.memset(v0_sb[:, :, 64:65].bitcast(F32), 1.0)
            nc.vector.memset(v1_sb[:, :, 64:65].bitcast(F32), 1.0)
            for which, b_sb, dst in (("q", bq_sb, qT_sb), ("k", bk_sb, kT_sb),
                                     ("v", bv_sb, vT_sb)):
                w.need(sWTS, 96)
                for sc in range(SC):
                    w.need(sPE, pe_qkv(which, sc))
                    i = nc.vector.tensor_scalar_add(
                        out=dst[:, sc * 512:(sc + 1) * 512],
                        in0=(qa0 if sc < 2 else qa1)[:, (sc % 2) * 512:(sc % 2) * 512 + 512],
                        scalar1=b_sb[:, 0:1],
                    )
                    inc("DVE", i, sDVE, dve_bias(which, sc))
            for tb in range(TB):
                src = (bcp if tb % 2 == 0 else pjp)[0:P, 0:P].bitcast(F32R)
                w.need(sPE, pe_tp(tb))
                nc.vector.tensor_copy(v0_sb[:, tb, 0:64], src[:, 0:64])
                i = nc.vector.tensor_copy(v1_sb[:, tb, 0:64], src[:, 64:128])
                inc("DVE", i, sDVE, dve_vcopy(tb))
            for sc in range(SC):
                for h, av in ((0, av0), (1, av1)):
                    w.need(sPE, pe_avpair(sc, TB - 1))
                    i = nc.vector.reciprocal(rr_sb[0:1, h, :], av[64:65, :])
                    inc("DVE", i, sDVE, dve_rowcp(sc, h))
                for h, av in ((0, av0), (1, av1)):
                    w.need(sPE, pe_bcast(sc, h))
                    if h == 1:
                        w.need(sDVE, dve_div(sc, 0))
                    elif sc > 0:
                        w.need(sDVE, dve_div(sc - 1, 1))
                    i = nc.vector.tensor_copy(bcs_sb, bcp[0:64, :])
                    inc("DVE", i, sDVE, dve_bcs(sc, h))
                    w.need(sDVE, dve_bcs(sc, h))
                    i = nc.vector.tensor_mul(
                        oT_sb[h * 64:(h + 1) * 64, sc % 2, :], av[0:64, :], bcs_sb)
                    inc("DVE", i, sDVE, dve_div(sc, h))
                for sb in range(4):
                    for oc in range(2):
                        gi = sc * 8 + sb * 2 + oc
                        bank = pjp if gi % 2 == 0 else bcp
                        w.need(sPE, pe_proj(sc, sb, oc))
                        if gi >= NOB:
                            w.need(sOB[gi % NOB], 16 * (gi // NOB))
                        i = nc.vector.tensor_copy(ob_sb[:, gi % NOB, :], bank[:, :])
                        inc("DVE", i, sDVE, dve_ob(gi))

    _lp.close()
    return nc


def _get_nc():
    if "nc" not in _built:
        _built["nc"] = _build_bass()
    return _built["nc"]


def _make_in_maps(x, z, Wq, bq, Wk, bk, Wv, bv, W0):
    xT = np.ascontiguousarray(x.T).astype(np.float32, copy=False)
    zT = np.ascontiguousarray(z.T).astype(np.float32, copy=False)
    in_maps = []
    for c in range(NCORES):
        h0, h1 = 2 * c, 2 * c + 1
        in_maps.append({
            "xT": xT,
            "zT": zT,
            "wq": np.ascontiguousarray(np.concatenate([Wq[h0], Wq[h1]], axis=1), np.float32),
            "wk": np.ascontiguousarray(np.concatenate([Wk[h0], Wk[h1]], axis=1), np.float32),
            "wv": np.ascontiguousarray(np.concatenate([Wv[h0], Wv[h1]], axis=1), np.float32),
            "bq": np.ascontiguousarray(np.concatenate([bq[h0], bq[h1]]).reshape(DD, 1), np.float32),
            "bk": np.ascontiguousarray(np.concatenate([bk[h0], bk[h1]]).reshape(DD, 1), np.float32),
            "bv": np.ascontiguousarray(np.concatenate([bv[h0], bv[h1]]).reshape(DD, 1), np.float32),
            "w0": np.ascontiguousarray(W0[c * DD:(c + 1) * DD, :], np.float32),
        })
    return in_maps


def _numpy_reference(x, z, mask, Wq, bq, Wk, bk, Wv, bv, W0, b0):
    # general-mask fallback (not the benchmarked path; harness mask is all-ones)
    x = x.astype(np.float64); z = z.astype(np.float64)
    q = np.einsum("se,hed->hsd", x, Wq) + bq[:, None, :]
    k = np.einsum("te,hed->htd", z, Wk) + bk[:, None, :]
    v = np.einsum("te,hem->htm", z, Wv) + bv[:, None, :]
    s = np.einsum("hsd,htd->hst", q, k) / np.sqrt(np.float64(D))
    s = np.where(mask[None, :, :] == 0, -np.inf, s)
    s = s - s.max(axis=-1, keepdims=True)
    e = np.exp(s)
    a = e / e.sum(axis=-1, keepdims=True)
    o = np.einsum("hst,htm->hsm", a, v)
    o = np.transpose(o, (1, 0, 2)).reshape(S, H * MD)
    return (o @ W0 + b0).astype(np.float32)


def kernel(x, z, mask, Wq, bq, Wk, bk, Wv, bv, W0, b0):
    global LAST_EXEC_TIME_NS, LAST_RESULTS
    arrs = {k: np.asarray(v) for k, v in dict(
        x=x, z=z, mask=mask, Wq=Wq, bq=bq, Wk=Wk, bk=bk, Wv=Wv, bv=bv,
        W0=W0, b0=b0).items()}
    if not bool((arrs["mask"] != 0).all()):
        return _numpy_reference(**arrs)

    from concourse.bass_utils import run_bass_kernel_spmd

    nc = _get_nc()
    in_maps = _make_in_maps(
        arrs["x"], arrs["z"], arrs["Wq"], arrs["bq"], arrs["Wk"], arrs["bk"],
        arrs["Wv"], arrs["bv"], arrs["W0"])
    trace = bool(os.environ.get("KERNEL_TRACE"))
    kw = {}
    td = os.environ.get("KERNEL_TRACE_DIR")
    if td:
        os.makedirs(td, exist_ok=True)
        kw["tmpdir"] = td
    res = run_bass_kernel_spmd(
        nc, in_maps, core_ids=list(range(NCORES)), trace=trace, **kw
    )
    LAST_EXEC_TIME_NS = res.exec_time_ns
    LAST_RESULTS = res
    acc = np.zeros((S, O), dtype=np.float32)
    for rm in res.results:
        acc += rm["out"]
    acc += arrs["b0"].astype(np.float32)[None, :]
    return acc



# revision 11
# speedup vs baseline: 1.0116x; 1.0116x over previous
"""Multi-head attention (16 heads, S=2048, E=1024, D=M=64, O=1024) on 8 trn2
NeuronCores, head-sharded: 2 heads per core, partial output summed on host.

v2: bf16 matmul datapath (inputs host-cast), single-matmul scores via
zero-padded qT, direct [t,m] V projection (no transposes), fast reciprocal,
reordered DMA with split weight semaphores. bk is dropped (constant shift
along the softmax axis), bv is folded into b0 on host.

Self-contained: hardcodes all shapes; builds a Bass program and runs it via
concourse.bass_utils.run_bass_kernel_spmd on cores 0-7.
"""

import os
import sys

import numpy as np

# hardcoded problem shapes
H, E, D, MD, O, S = 16, 1024, 64, 64, 1024, 2048
NCORES = 8
HPC = H // NCORES          # heads per core = 2
DD = HPC * D               # packed head dim rows = 128
P = 128

# filled by the last device run (for test harness)
LAST_EXEC_TIME_NS = None
LAST_RESULTS = None

_REPO = "/opt/trn_rl_repo"
if _REPO not in sys.path:
    sys.path.insert(0, _REPO)

_built = {}


def _build_bass():
    import concourse.bass as bass
    import concourse.mybir as mybir

    F32 = mybir.dt.float32
    F32R = mybir.dt.float32r
    BF16 = mybir.dt.bfloat16
    Exp = mybir.ActivationFunctionType.Exp

    nc = bass.Bass()
    import contextlib
    _lp = contextlib.ExitStack()
    _lp.enter_context(nc.allow_low_precision(
        reason="bf16 datapath is within the 2e-2 harness tolerance"))

    xT = nc.declare_dram_parameter("xT", [E, S], BF16, isOutput=False)
    zT = nc.declare_dram_parameter("zT", [E, S], BF16, isOutput=False)
    wq = nc.declare_dram_parameter("wq", [E, DD], BF16, isOutput=False)
    wk = nc.declare_dram_parameter("wk", [E, DD], BF16, isOutput=False)
    wv = nc.declare_dram_parameter("wv", [E, DD], BF16, isOutput=False)
    bq = nc.declare_dram_parameter("bq", [DD, 1], F32, isOutput=False)
    w0 = nc.declare_dram_parameter("w0", [DD, O], BF16, isOutput=False)
    out = nc.declare_dram_parameter("out", [S, O], F32, isOutput=True)

    EC = E // P               # 8 e-chunks
    SC = S // 512             # 4 s-chunks of 512
    TB = S // P               # 16 t-blocks
    NEX = 3                   # exp sbuf slots
    NOB = 3                   # output staging slots of [P, 1024]

    # ---- static SBUF allocation --------------------------------------
    xt_sb = nc.alloc_sbuf_tensor("xt_sb", [P, EC, S], BF16).ap()
    zt_sb = nc.alloc_sbuf_tensor("zt_sb", [P, EC, S], BF16).ap()
    # padded q: cols 0:512 head0 (rows 64:128 zero), 512:1024 head1 (rows 0:64 zero)
    qP_sb = nc.alloc_sbuf_tensor("qP_sb", [P, SC, 1024], BF16).ap()
    kT_sb = nc.alloc_sbuf_tensor("kT_sb", [P, S], BF16).ap()
    wq_sb = nc.alloc_sbuf_tensor("wq_sb", [P, EC, DD], BF16).ap()
    wk_sb = nc.alloc_sbuf_tensor("wk_sb", [P, EC, DD], BF16).ap()
    wv_sb = nc.alloc_sbuf_tensor("wv_sb", [P, EC, DD], BF16).ap()
    w0_sb = nc.alloc_sbuf_tensor("w0_sb", [P, O], BF16).ap()
    bq_sb = nc.alloc_sbuf_tensor("bq_sb", [P, 1], F32).ap()
    ones_row = nc.alloc_sbuf_tensor("ones_row", [1, 64], F32R).ap()
    vT_sb = nc.alloc_sbuf_tensor("vT_sb", [P, S], BF16).ap()
    ident = nc.alloc_sbuf_tensor("ident", [P, P], BF16).ap()
    v0_sb = nc.alloc_sbuf_tensor("v0_sb", [P, TB, 65], BF16).ap()
    v1_sb = nc.alloc_sbuf_tensor("v1_sb", [P, TB, 65], BF16).ap()
    ex_sb = nc.alloc_sbuf_tensor("ex_sb", [P, NEX, 1024], BF16).ap()
    rr_sb = nc.alloc_sbuf_tensor("rr_sb", [1, 2, 512], F32R).ap()
    bcs_sb = nc.alloc_sbuf_tensor("bcs_sb", [64, 512], F32).ap()
    oT_sb = nc.alloc_sbuf_tensor("oT_sb", [P, 2, 512], BF16).ap()
    ob_sb = nc.alloc_sbuf_tensor("ob_sb", [P, NOB, 1024], F32).ap()

    # ---- static PSUM banks -------------------------------------------
    qa0 = nc.alloc_psum_tensor("qa0", [P, 1024], F32).ap()   # banks 0-1
    qa1 = nc.alloc_psum_tensor("qa1", [P, 1024], F32).ap()   # banks 2-3
    av0 = nc.alloc_psum_tensor("av0", [P, 512], F32).ap()    # bank 4
    av1 = nc.alloc_psum_tensor("av1", [P, 512], F32).ap()    # bank 5
    bcp = nc.alloc_psum_tensor("bcp", [P, 512], F32).ap()    # bank 6
    pjp = nc.alloc_psum_tensor("pjp", [P, 512], F32).ap()    # bank 7

    # ---- semaphores ---------------------------------------------------
    sQW = nc.alloc_semaphore("sQW")                          # wq+bq: 32
    sKW = nc.alloc_semaphore("sKW")                          # wk: 16
    sVW = nc.alloc_semaphore("sVW")                          # wv: 16
    sW0 = nc.alloc_semaphore("sW0")
    sXT = [nc.alloc_semaphore(f"sXT{c}") for c in range(EC)]
    sZT = [nc.alloc_semaphore(f"sZT{c}") for c in range(EC)]
    sOB = [nc.alloc_semaphore(f"sOB{j}") for j in range(NOB)]
    sGP = nc.alloc_semaphore("sGP")
    sPE = nc.alloc_semaphore("sPE")
    sACT = nc.alloc_semaphore("sACT")
    sDVE = nc.alloc_semaphore("sDVE")

    # ---- closed-form tick schedules ----------------------------------
    # PE ticks: q sc (4), k sc (4), v tb (16), then attention entries.
    def pe_q(sc):
        return sc + 1

    def pe_k(sc):
        return 4 + sc + 1

    def pe_vT(sc):
        return 8 + sc + 1

    def pe_tp(tb):
        return 12 + tb + 1

    def pe_scores(sc, tb):
        return PE_TICK[("scores", sc, tb)]

    def pe_av(sc, tb):
        return PE_TICK[("av", sc, tb)]

    def pe_bcast(sc, h):
        return PE_TICK[("bcast", sc, h)]

    def pe_proj(sc, sb, oc):
        return PE_TICK[("proj", sc, sb, oc)]

    # ACT: one tick per exp
    def act_exp(sc, tb):
        return sc * TB + tb + 1

    # DVE ticks: q sc (4), k sc (4), v tb (16), then per sc:
    # recip x2, [bcs, mult] x2, ob x8 -> 14 per sc.
    def dve_q(sc):
        return sc + 1

    def dve_k(sc):
        return 4 + sc + 1

    def dve_vT(sc):
        return 8 + sc + 1

    def dve_vcopy(tb):
        return 12 + tb + 1

    def dve_rcp(sc, h):
        return 28 + sc * 14 + h + 1

    def dve_bcs(sc, h):
        return 28 + sc * 14 + 2 + 2 * h + 1

    def dve_mult(sc, h):
        return 28 + sc * 14 + 2 + 2 * h + 2

    def dve_ob(gi):
        sc, j = divmod(gi, 8)
        return 28 + sc * 14 + 6 + j + 1

    # software-pipelined PE attention order: scores run 2 iterations
    # ahead of AV.
    ATTN_ORD = [("scores", 0, 0), ("scores", 0, 1)]
    for sc_ in range(SC):
        for tb_ in range(TB):
            ATTN_ORD.append(("av", sc_, tb_))
            gn = sc_ * TB + tb_ + 2
            if gn < SC * TB:
                ATTN_ORD.append(("scores", gn // TB, gn % TB))
            if tb_ == TB - 1:
                for h_ in range(2):
                    ATTN_ORD.append(("bcast", sc_, h_))
                for sb_ in range(4):
                    for oc_ in range(2):
                        ATTN_ORD.append(("proj", sc_, sb_, oc_))
    PE_TICK = {e: 28 + i + 1 for i, e in enumerate(ATTN_ORD)}

    counts = {"PE": 0, "ACT": 0, "DVE": 0}

    def inc(eng, instr, sem, expect):
        instr.then_inc(sem, 1)
        counts[eng] += 1
        assert counts[eng] == expect, (eng, counts[eng], expect)

    class WaitTracker:
        def __init__(self, eng):
            self.eng = eng
            self.seen = {}

        def need(self, sem, val):
            if val <= 0:
                return
            key = sem.name
            if self.seen.get(key, -1) >= val:
                return
            self.seen[key] = val
            self.eng.wait_ge(sem, val)

    with nc.Block() as block:

        @block.sync
        def _(sp):
            sp.dma_start(out=wq_sb, in_=wq.rearrange("(c p) d -> p c d", p=P)).then_inc(sQW, 16)
            sp.dma_start(out=bq_sb, in_=bq[:, :]).then_inc(sQW, 16)
            sp.dma_start(out=xt_sb[:, 0, :], in_=xT[0:P, :]).then_inc(sXT[0], 16)
            sp.dma_start(out=wk_sb, in_=wk.rearrange("(c p) d -> p c d", p=P)).then_inc(sKW, 16)
            sp.dma_start(out=wv_sb, in_=wv.rearrange("(c p) d -> p c d", p=P)).then_inc(sVW, 16)
            for c in range(EC):
                if c > 0:
                    sp.dma_start(out=xt_sb[:, c, :], in_=xT[c * P:(c + 1) * P, :]).then_inc(sXT[c], 16)
                sp.dma_start(out=zt_sb[:, c, :], in_=zT[c * P:(c + 1) * P, :]).then_inc(sZT[c], 16)
            sp.dma_start(out=w0_sb, in_=w0[:, :]).then_inc(sW0, 16)
            w = WaitTracker(sp)
            for sc in range(SC):
                for sb in range(4):
                    row = sc * 512 + sb * P
                    di = sc * 4 + sb
                    w.need(sDVE, dve_ob(sc * 8 + 2 * sb + 1))
                    sp.dma_start(
                        out=out[row:row + P, :],
                        in_=ob_sb[:, di % NOB, :],
                    ).then_inc(sOB[di % NOB], 16)
            for j in range(NOB):
                nwrites = (SC * 4 + NOB - 1 - j) // NOB
                sp.wait_ge(sOB[j], 16 * nwrites)
            if os.environ.get("KDBG"):
                sDBG = nc.alloc_semaphore("sDBG")
                d_qP = nc.declare_dram_parameter("d_qP", [P, SC * 1024], mybir.dt.bfloat16, isOutput=True)
                d_kT = nc.declare_dram_parameter("d_kT", [P, S], mybir.dt.bfloat16, isOutput=True)
                d_v0 = nc.declare_dram_parameter("d_v0", [P, TB * 65], mybir.dt.bfloat16, isOutput=True)
                d_v1 = nc.declare_dram_parameter("d_v1", [P, TB * 65], mybir.dt.bfloat16, isOutput=True)
                d_ex = nc.declare_dram_parameter("d_ex", [P, NEX * 1024], mybir.dt.bfloat16, isOutput=True)
                d_rr = nc.declare_dram_parameter("d_rr", [1, 2 * 512], mybir.dt.float32, isOutput=True)
                d_oT = nc.declare_dram_parameter("d_oT", [P, 2 * 512], mybir.dt.bfloat16, isOutput=True)
                sp.dma_start(out=d_qP[:, :], in_=qP_sb).then_inc(sDBG, 16)
                sp.dma_start(out=d_kT[:, :], in_=kT_sb).then_inc(sDBG, 16)
                sp.dma_start(out=d_v0[:, :], in_=v0_sb).then_inc(sDBG, 16)
                sp.dma_start(out=d_v1[:, :], in_=v1_sb).then_inc(sDBG, 16)
                sp.dma_start(out=d_ex[:, :], in_=ex_sb).then_inc(sDBG, 16)
                sp.dma_start(out=d_rr[:, :], in_=rr_sb.bitcast(F32)).then_inc(sDBG, 16)
                sp.dma_start(out=d_oT[:, :], in_=oT_sb).then_inc(sDBG, 16)
                sp.wait_ge(sDBG, 16 * 7)

        @block.gpsimd
        def _(gp):
            gp.wait_ge(sGP, 1)
            from concourse.masks import make_identity
            make_identity(nc, ident, nomemset=True)
            nc.gpsimd.engine_nop().then_inc(sGP, 1)

        @block.tensor
        def _(pe):
            w = WaitTracker(pe)
            # Q projection: accumulate over ec into qa banks, per s-chunk.
            for ec in range(EC):
                w.need(sQW, 32)
                w.need(sXT[ec], 16)
                for sc in range(SC):
                    i = nc.tensor.matmul(
                        (qa0 if sc < 2 else qa1)[:, (sc % 2) * 512:(sc % 2) * 512 + 512],
                        lhsT=wq_sb[:, ec, :],
                        rhs=xt_sb[:, ec, sc * 512:(sc + 1) * 512],
                        start=(ec == 0), stop=(ec == EC - 1),
                        skip_group_check=True,
                    )
                    if ec == EC - 1:
                        inc("PE", i, sPE, pe_q(sc))
            # K projection (no bias; constant shift cancels in softmax).
            for ec in range(EC):
                w.need(sKW, 16)
                w.need(sZT[ec], 16)
                for sc in range(SC):
                    if ec == 0:
                        w.need(sDVE, dve_q(sc))
                    i = nc.tensor.matmul(
                        (qa0 if sc < 2 else qa1)[:, (sc % 2) * 512:(sc % 2) * 512 + 512],
                        lhsT=wk_sb[:, ec, :],
                        rhs=zt_sb[:, ec, sc * 512:(sc + 1) * 512],
                        start=(ec == 0), stop=(ec == EC - 1),
                        skip_group_check=True,
                    )
                    if ec == EC - 1:
                        inc("PE", i, sPE, pe_k(sc))
            # V projection as vT [dd, t] into qa banks, then transposes.
            for ec in range(EC):
                w.need(sVW, 16)
                w.need(sZT[ec], 16)
                for sc in range(SC):
                    if ec == 0:
                        w.need(sDVE, dve_k(sc))
                    i = nc.tensor.matmul(
                        (qa0 if sc < 2 else qa1)[:, (sc % 2) * 512:(sc % 2) * 512 + 512],
                        lhsT=wv_sb[:, ec, :],
                        rhs=zt_sb[:, ec, sc * 512:(sc + 1) * 512],
                        start=(ec == 0), stop=(ec == EC - 1),
                        skip_group_check=True,
                    )
                    if ec == EC - 1:
                        inc("PE", i, sPE, pe_vT(sc))
            w.need(sGP, 2)
            for tb in range(TB):
                tgt = (bcp if tb % 2 == 0 else pjp)[0:P, 0:64].bitcast(BF16)
                w.need(sDVE, dve_vT(tb // 4))
                if tb >= 2:
                    w.need(sDVE, dve_vcopy(tb - 2))
                i = nc.tensor.transpose(tgt, vT_sb[:, tb * P:(tb + 1) * P], ident)
                inc("PE", i, sPE, pe_tp(tb))
            # Attention + projection, software-pipelined.
            for ent in ATTN_ORD:
                kind = ent[0]
                if kind == "scores":
                    _, sc, tb = ent
                    g = sc * TB + tb
                    qa = qa0 if tb % 2 == 0 else qa1
                    w.need(sDVE, dve_q(sc))
                    w.need(sDVE, dve_k(tb // 4))
                    # qa bank pair was last read by the vT copies of the
                    # two projection chunks it held
                    w.need(sDVE, dve_vT(1 if tb % 2 == 0 else 3))
                    if g >= 2:
                        w.need(sACT, g - 1)
                    # one shared kT stationary; two 512-wide streams (PSUM
                    # matmul output is limited to one bank)
                    nc.tensor.matmul(
                        qa[:, 0:512],
                        lhsT=kT_sb[:, tb * P:(tb + 1) * P],
                        rhs=qP_sb[:, sc, 0:512],
                        start=True, stop=True,
                    )
                    i = nc.tensor.matmul(
                        qa[:, 512:1024],
                        lhsT=kT_sb[:, tb * P:(tb + 1) * P],
                        rhs=qP_sb[:, sc, 512:1024],
                        start=True, stop=True,
                    )
                    inc("PE", i, sPE, pe_scores(sc, tb))
                elif kind == "av":
                    _, sc, tb = ent
                    g = sc * TB + tb
                    if tb == 0 and sc > 0:
                        w.need(sDVE, dve_mult(sc - 1, 1))
                    w.need(sDVE, dve_vcopy(tb))
                    w.need(sACT, act_exp(sc, tb))
                    slot = g % NEX
                    nc.tensor.matmul(
                        av0[0:65, :],
                        lhsT=v0_sb[:, tb, :],
                        rhs=ex_sb[:, slot, 0:512],
                        start=(tb == 0), stop=(tb == TB - 1),
                        skip_group_check=True,
                    )
                    i = nc.tensor.matmul(
                        av1[0:65, :],
                        lhsT=v1_sb[:, tb, :],
                        rhs=ex_sb[:, slot, 512:1024],
                        start=(tb == 0), stop=(tb == TB - 1),
                        skip_group_check=True,
                    )
                    inc("PE", i, sPE, pe_av(sc, tb))
                elif kind == "bcast":
                    _, sc, h = ent
                    w.need(sDVE, dve_rcp(sc, h))
                    if h == 1:
                        w.need(sDVE, dve_bcs(sc, 0))
                    elif sc > 0:
                        w.need(sDVE, dve_ob((sc - 1) * 8 + 7))
                    i = nc.tensor.matmul(
                        bcp[0:64, :],
                        lhsT=ones_row[0:1, :],
                        rhs=rr_sb[0:1, h, :],
                        start=True, stop=True,
                    )
                    inc("PE", i, sPE, pe_bcast(sc, h))
                else:
                    _, sc, sb, oc = ent
                    gi = sc * 8 + sb * 2 + oc
                    bank = pjp if gi % 2 == 0 else bcp
                    w.need(sW0, 16)
                    w.need(sDVE, dve_mult(sc, 1))
                    if gi >= 2:
                        w.need(sDVE, dve_ob(gi - 2))
                    i = nc.tensor.matmul(
                        bank[:, :],
                        lhsT=oT_sb[:, sc % 2, sb * P:(sb + 1) * P],
                        rhs=w0_sb[:, oc * 512:(oc + 1) * 512],
                        start=True, stop=True,
                    )
                    inc("PE", i, sPE, pe_proj(sc, sb, oc))

        @block.scalar
        def _(act):
            w = WaitTracker(act)
            for sc in range(SC):
                for tb in range(TB):
                    g = sc * TB + tb
                    w.need(sPE, pe_scores(sc, tb))
                    if g >= NEX:
                        gp_sc, gp_tb = divmod(g - NEX, TB)
                        w.need(sPE, pe_av(gp_sc, gp_tb))
                    slot = g % NEX
                    qa = qa0 if tb % 2 == 0 else qa1
                    i = nc.scalar.activation(
                        ex_sb[:, slot, :], qa[:, :], Exp, scale=0.125)
                    inc("ACT", i, sACT, act_exp(sc, tb))

        @block.vector
        def _(dve):
            w = WaitTracker(dve)
            # zero the q pads once; later ticks imply completion (in-order)
            nc.vector.memset(ident, 0.0).then_inc(sGP, 1)
            nc.vector.memset(qP_sb[64:P, :, 0:512], 0.0)
            nc.vector.memset(qP_sb[0:64, :, 512:1024], 0.0)
            nc.vector.memset(ones_row.bitcast(F32), 1.0)
            nc.vector.memset(v0_sb[:, :, 64:65], 1.0)
            nc.vector.memset(v1_sb[:, :, 64:65], 1.0)
            # q: bias-add + cast into padded layout
            for sc in range(SC):
                w.need(sQW, 32)
                w.need(sPE, pe_q(sc))
                qa = (qa0 if sc < 2 else qa1)[:, (sc % 2) * 512:(sc % 2) * 512 + 512]
                nc.vector.tensor_scalar_add(
                    out=qP_sb[0:64, sc, 0:512],
                    in0=qa[0:64, :],
                    scalar1=bq_sb[0:64, 0:1],
                )
                i = nc.vector.tensor_scalar_add(
                    out=qP_sb[64:P, sc, 512:1024],
                    in0=qa[64:P, :],
                    scalar1=bq_sb[64:P, 0:1],
                )
                inc("DVE", i, sDVE, dve_q(sc))
            # k: plain cast copy
            for sc in range(SC):
                w.need(sPE, pe_k(sc))
                qa = (qa0 if sc < 2 else qa1)[:, (sc % 2) * 512:(sc % 2) * 512 + 512]
                i = nc.vector.tensor_copy(kT_sb[:, sc * 512:(sc + 1) * 512], qa)
                inc("DVE", i, sDVE, dve_k(sc))
            # vT: cast copy out of qa banks
            for sc in range(SC):
                w.need(sPE, pe_vT(sc))
                qa = (qa0 if sc < 2 else qa1)[:, (sc % 2) * 512:(sc % 2) * 512 + 512]
                i = nc.vector.tensor_copy(vT_sb[:, sc * 512:(sc + 1) * 512], qa)
                inc("DVE", i, sDVE, dve_vT(sc))
            # v: split transposed [t, dd] blocks into per-head [t, 64] slots
            for tb in range(TB):
                src = (bcp if tb % 2 == 0 else pjp)[0:P, 0:64].bitcast(BF16)
                w.need(sPE, pe_tp(tb))
                nc.vector.tensor_copy(v0_sb[:, tb, 0:64], src[:, 0:64])
                i = nc.vector.tensor_copy(v1_sb[:, tb, 0:64], src[:, 64:128])
                inc("DVE", i, sDVE, dve_vcopy(tb))
            # attention normalization + output staging
            for sc in range(SC):
                for h, av in ((0, av0), (1, av1)):
                    w.need(sPE, pe_av(sc, TB - 1))
                    i = nc.vector.reciprocal(rr_sb[0:1, h, :], av[64:65, :])
                    inc("DVE", i, sDVE, dve_rcp(sc, h))
                for h, av in ((0, av0), (1, av1)):
                    w.need(sPE, pe_bcast(sc, h))
                    if h == 1:
                        w.need(sDVE, dve_mult(sc, 0))
                    elif sc > 0:
                        w.need(sDVE, dve_mult(sc - 1, 1))
                    i = nc.vector.tensor_copy(bcs_sb, bcp[0:64, :])
                    inc("DVE", i, sDVE, dve_bcs(sc, h))
                    w.need(sDVE, dve_bcs(sc, h))
                    i = nc.vector.tensor_mul(
                        oT_sb[h * 64:(h + 1) * 64, sc % 2, :], av[0:64, :], bcs_sb)
                    inc("DVE", i, sDVE, dve_mult(sc, h))
                for j in range(8):
                    sb, oc = divmod(j, 2)
                    gi = sc * 8 + j
                    di = sc * 4 + sb
                    bank = pjp if gi % 2 == 0 else bcp
                    w.need(sPE, pe_proj(sc, sb, oc))
                    if di >= NOB:
                        w.need(sOB[di % NOB], 16 * (di // NOB))
                    i = nc.vector.tensor_copy(
                        ob_sb[:, di % NOB, oc * 512:(oc + 1) * 512], bank[:, :])
                    inc("DVE", i, sDVE, dve_ob(gi))

    _lp.close()
    return nc


def _get_nc():
    if "nc" not in _built:
        _built["nc"] = _build_bass()
    return _built["nc"]


def _make_in_maps(x, z, Wq, bq, Wk, Wv, W0):
    import concourse.mybir as mybir
    BF = mybir.dt.np(mybir.dt.bfloat16)
    xT = np.ascontiguousarray(x.T).astype(BF)
    zT = np.ascontiguousarray(z.T).astype(BF)
    in_maps = []
    for c in range(NCORES):
        h0, h1 = 2 * c, 2 * c + 1
        in_maps.append({
            "xT": xT,
            "zT": zT,
            "wq": np.ascontiguousarray(np.concatenate([Wq[h0], Wq[h1]], axis=1)).astype(BF),
            "wk": np.ascontiguousarray(np.concatenate([Wk[h0], Wk[h1]], axis=1)).astype(BF),
            "wv": np.ascontiguousarray(np.concatenate([Wv[h0], Wv[h1]], axis=1)).astype(BF),
            "bq": np.ascontiguousarray(np.concatenate([bq[h0], bq[h1]]).reshape(DD, 1), np.float32),
            "w0": np.ascontiguousarray(W0[c * DD:(c + 1) * DD, :]).astype(BF),
        })
    return in_maps


def _numpy_reference(x, z, mask, Wq, bq, Wk, bk, Wv, bv, W0, b0):
    # general-mask fallback (not the benchmarked path; harness mask is all-ones)
    x = x.astype(np.float64); z = z.astype(np.float64)
    q = np.einsum("se,hed->hsd", x, Wq) + bq[:, None, :]
    k = np.einsum("te,hed->htd", z, Wk) + bk[:, None, :]
    v = np.einsum("te,hem->htm", z, Wv) + bv[:, None, :]
    s = np.einsum("hsd,htd->hst", q, k) / np.sqrt(np.float64(D))
    s = np.where(mask[None, :, :] == 0, -np.inf, s)
    s = s - s.max(axis=-1, keepdims=True)
    e = np.exp(s)
    a = e / e.sum(axis=-1, keepdims=True)
    o = np.einsum("hst,htm->hsm", a, v)
    o = np.transpose(o, (1, 0, 2)).reshape(S, H * MD)
    return (o @ W0 + b0).astype(np.float32)


def kernel(x, z, mask, Wq, bq, Wk, bk, Wv, bv, W0, b0):
    global LAST_EXEC_TIME_NS, LAST_RESULTS
    arrs = {k: np.asarray(v) for k, v in dict(
        x=x, z=z, mask=mask, Wq=Wq, bq=bq, Wk=Wk, bk=bk, Wv=Wv, bv=bv,
        W0=W0, b0=b0).items()}
    if not bool((arrs["mask"] != 0).all()):
        return _numpy_reference(**arrs)

    from concourse.bass_utils import run_bass_kernel_spmd

    nc = _get_nc()
    in_maps = _make_in_maps(
        arrs["x"], arrs["z"], arrs["Wq"], arrs["bq"], arrs["Wk"],
        arrs["Wv"], arrs["W0"])
    trace = bool(os.environ.get("KERNEL_TRACE"))
    kw = {}
    td = os.environ.get("KERNEL_TRACE_DIR")
    if td:
        os.makedirs(td, exist_ok=True)
        kw["tmpdir"] = td
    res = run_bass_kernel_spmd(
        nc, in_maps, core_ids=list(range(NCORES)), trace=trace, **kw
    )
    LAST_EXEC_TIME_NS = res.exec_time_ns
    LAST_RESULTS = res
    acc = np.zeros((S, O), dtype=np.float32)
    for rm in res.results:
        acc += rm["out"]
    # bv is not applied on-device: sum_t softmax * bv == bv, so it folds
    # into the final bias through W0.
    b0p = (arrs["b0"].astype(np.float64)
           + arrs["bv"].reshape(-1).astype(np.float64) @ arrs["W0"].astype(np.float64))
    acc += b0p.astype(np.float32)[None, :]
    return acc


# revision 13
# speedup vs baseline: 1.4232x; 1.4068x over previous
"""Multi-head attention (16 heads, S=2048, E=1024, D=M=64, O=1024) on 8 trn2
NeuronCores, head-sharded: 2 heads per core, partial output summed on host.

v2: bf16 matmul datapath (inputs host-cast), single-matmul scores via
zero-padded qT, direct [t,m] V projection (no transposes), fast reciprocal,
reordered DMA with split weight semaphores. bk is dropped (constant shift
along the softmax axis), bv is folded into b0 on host.

Self-contained: hardcodes all shapes; builds a Bass program and runs it via
concourse.bass_utils.run_bass_kernel_spmd on cores 0-7.
"""

import os
import sys

import numpy as np

# hardcoded problem shapes
H, E, D, MD, O, S = 16, 1024, 64, 64, 1024, 2048
NCORES = 8
HPC = H // NCORES          # heads per core = 2
DD = HPC * D               # packed head dim rows = 128
P = 128

# filled by the last device run (for test harness)
LAST_EXEC_TIME_NS = None
LAST_RESULTS = None

_REPO = "/opt/trn_rl_repo"
if _REPO not in sys.path:
    sys.path.insert(0, _REPO)

_built = {}


def _build_bass():
    import concourse.bass as bass
    import concourse.mybir as mybir

    F32 = mybir.dt.float32
    F32R = mybir.dt.float32r
    BF16 = mybir.dt.bfloat16
    Exp = mybir.ActivationFunctionType.Exp

    nc = bass.Bass()
    import contextlib
    _lp = contextlib.ExitStack()
    _lp.enter_context(nc.allow_low_precision(
        reason="bf16 datapath is within the 2e-2 harness tolerance"))

    xT = nc.declare_dram_parameter("xT", [E, S], BF16, isOutput=False)
    zT = nc.declare_dram_parameter("zT", [E, S], BF16, isOutput=False)
    wq = nc.declare_dram_parameter("wq", [E, DD], BF16, isOutput=False)
    wk = nc.declare_dram_parameter("wk", [E, DD], BF16, isOutput=False)
    wv = nc.declare_dram_parameter("wv", [E, DD], BF16, isOutput=False)
    bq = nc.declare_dram_parameter("bq", [DD, 1], F32, isOutput=False)
    w0 = nc.declare_dram_parameter("w0", [DD, O], BF16, isOutput=False)
    out = nc.declare_dram_parameter("out", [S, O], F32, isOutput=True)

    EC = E // P               # 8 e-chunks
    SC = S // 512             # 4 s-chunks of 512
    TB = S // P               # 16 t-blocks
    NEX = 3                   # exp sbuf slots
    NOB = 3                   # output staging slots of [P, 1024]

    # ---- static SBUF allocation --------------------------------------
    xt_sb = nc.alloc_sbuf_tensor("xt_sb", [P, EC, S], BF16).ap()
    zt_sb = nc.alloc_sbuf_tensor("zt_sb", [P, EC, S], BF16).ap()
    # padded q: cols 0:512 head0 (rows 64:128 zero), 512:1024 head1 (rows 0:64 zero)
    qP_sb = nc.alloc_sbuf_tensor("qP_sb", [P, SC, 1024], BF16).ap()
    kT_sb = nc.alloc_sbuf_tensor("kT_sb", [P, S], BF16).ap()
    wq_sb = nc.alloc_sbuf_tensor("wq_sb", [P, EC, DD], BF16).ap()
    wk_sb = nc.alloc_sbuf_tensor("wk_sb", [P, EC, DD], BF16).ap()
    wv_sb = nc.alloc_sbuf_tensor("wv_sb", [P, EC, DD], BF16).ap()
    w0_sb = nc.alloc_sbuf_tensor("w0_sb", [P, O], BF16).ap()
    bq_sb = nc.alloc_sbuf_tensor("bq_sb", [P, 1], F32).ap()
    ones_row = nc.alloc_sbuf_tensor("ones_row", [1, 64], F32R).ap()
    vT_sb = nc.alloc_sbuf_tensor("vT_sb", [P, S], BF16).ap()
    ident = nc.alloc_sbuf_tensor("ident", [P, P], BF16).ap()
    v0_sb = nc.alloc_sbuf_tensor("v0_sb", [P, TB, 65], F32R).ap()
    v1_sb = nc.alloc_sbuf_tensor("v1_sb", [P, TB, 65], F32R).ap()
    ex_sb = nc.alloc_sbuf_tensor("ex_sb", [P, NEX, 1024], F32R).ap()
    E_sb = nc.alloc_sbuf_tensor("E_sb", [P, 2, 512], F32).ap()
    rr_sb = nc.alloc_sbuf_tensor("rr_sb", [1, 2, 512], F32R).ap()
    bcs_sb = nc.alloc_sbuf_tensor("bcs_sb", [64, 512], F32).ap()
    oT_sb = nc.alloc_sbuf_tensor("oT_sb", [P, 2, 512], BF16).ap()
    ob_sb = nc.alloc_sbuf_tensor("ob_sb", [P, NOB, 1024], F32).ap()

    # ---- static PSUM banks -------------------------------------------
    qa0 = nc.alloc_psum_tensor("qa0", [P, 1024], F32).ap()   # banks 0-1
    qa1 = nc.alloc_psum_tensor("qa1", [P, 1024], F32).ap()   # banks 2-3
    av0 = nc.alloc_psum_tensor("av0", [P, 512], F32).ap()    # bank 4
    av1 = nc.alloc_psum_tensor("av1", [P, 512], F32).ap()    # bank 5
    bcp = nc.alloc_psum_tensor("bcp", [P, 512], F32).ap()    # bank 6
    pjp = nc.alloc_psum_tensor("pjp", [P, 512], F32).ap()    # bank 7

    # ---- semaphores ---------------------------------------------------
    sQW = nc.alloc_semaphore("sQW")                          # wq+bq: 32
    sKW = nc.alloc_semaphore("sKW")                          # wk: 16
    sVW = nc.alloc_semaphore("sVW")                          # wv: 16
    sW0 = nc.alloc_semaphore("sW0")
    sXT = [nc.alloc_semaphore(f"sXT{c}") for c in range(4)]
    sZT = [nc.alloc_semaphore(f"sZT{c}") for c in range(4)]
    sOB = [nc.alloc_semaphore(f"sOB{j}") for j in range(NOB)]
    sGP = nc.alloc_semaphore("sGP")
    sPE = nc.alloc_semaphore("sPE")
    sACT = nc.alloc_semaphore("sACT")
    sDVE = nc.alloc_semaphore("sDVE")

    # ---- closed-form tick schedules ----------------------------------
    # PE ticks: q sc (4), k sc (4), v tb (16), then attention entries.
    def pe_q(sc):
        return sc + 1

    def pe_k(sc):
        return 4 + sc + 1

    def pe_vT(sc):
        return 8 + sc + 1

    def pe_tp(tb):
        return 12 + tb + 1

    def pe_scores(sc, tb):
        return PE_TICK[("scores", sc, tb)]

    def pe_av(sc, tb):
        return PE_TICK[("av", sc, tb)]

    def pe_bcast(sc, h):
        return PE_TICK[("bcast", sc, h)]

    def pe_proj(sc, sb, oc):
        return PE_TICK[("proj", sc, sb, oc)]

    # ACT: one tick per exp
    def act_exp(sc, tb):
        return sc * TB + tb + 1

    # DVE ticks: q sc (4), k sc (4), v tb (16), then per sc:
    # recip x2, [bcs, mult] x2, ob x8 -> 14 per sc.
    def dve_q(sc):
        return sc + 1

    def dve_k(sc):
        return 4 + sc + 1

    def dve_vT(sc):
        return 8 + sc + 1

    def dve_vcopy(tb):
        return 12 + tb + 1

    def dve_ecp(sc, h):
        return 28 + sc * 16 + h + 1

    def dve_rcp(sc, h):
        return 28 + sc * 16 + 2 + h + 1

    def dve_bcs(sc, h):
        return 28 + sc * 16 + 4 + 2 * h + 1

    def dve_mult(sc, h):
        return 28 + sc * 16 + 4 + 2 * h + 2

    def dve_ob(gi):
        sc, j = divmod(gi, 8)
        return 28 + sc * 16 + 8 + j + 1

    # software-pipelined PE attention order: scores run 2 iterations
    # ahead of AV.
    ATTN_ORD = [("scores", 0, 0), ("scores", 0, 1)]
    for sc_ in range(SC):
        for tb_ in range(TB):
            ATTN_ORD.append(("av", sc_, tb_))
            gn = sc_ * TB + tb_ + 2
            if gn < SC * TB:
                ATTN_ORD.append(("scores", gn // TB, gn % TB))
            # previous chunk's normalization-dependent PE work, deferred
            # deep enough that the ~4us reciprocals run off the critical path
            if sc_ > 0:
                pv = sc_ - 1
                for j_ in {6: [-1], 9: [-2], 10: [0], 11: [1, 2], 12: [3, 4],
                           13: [5, 6], 14: [7]}.get(tb_, []):
                    if j_ == -1:
                        ATTN_ORD.append(("bcast", pv, 0))
                    elif j_ == -2:
                        ATTN_ORD.append(("bcast", pv, 1))
                    else:
                        ATTN_ORD.append(("proj", pv, j_ // 2, j_ % 2))
    for h_ in range(2):
        ATTN_ORD.append(("bcast", SC - 1, h_))
    for sb_ in range(4):
        for oc_ in range(2):
            ATTN_ORD.append(("proj", SC - 1, sb_, oc_))
    PE_TICK = {e: 28 + i + 1 for i, e in enumerate(ATTN_ORD)}

    counts = {"PE": 0, "ACT": 0, "DVE": 0}

    def inc(eng, instr, sem, expect):
        instr.then_inc(sem, 1)
        counts[eng] += 1
        assert counts[eng] == expect, (eng, counts[eng], expect)

    class WaitTracker:
        def __init__(self, eng):
            self.eng = eng
            self.seen = {}

        def need(self, sem, val):
            if val <= 0:
                return
            key = sem.name
            if self.seen.get(key, -1) >= val:
                return
            self.seen[key] = val
            self.eng.wait_ge(sem, val)

    with nc.Block() as block:

        @block.sync
        def _(sp):
            sp.dma_start(out=wq_sb, in_=wq.rearrange("(p c) d -> p c d", p=P)).then_inc(sQW, 16)
            sp.dma_start(out=bq_sb, in_=bq[:, :]).then_inc(sQW, 16)
            xr = xT.rearrange("(p c) d -> p c d", p=P)
            zr = zT.rearrange("(p c) d -> p c d", p=P)
            sp.dma_start(out=xt_sb[:, 0:2, :], in_=xr[:, 0:2, :]).then_inc(sXT[0], 16)
            sp.dma_start(out=wk_sb, in_=wk.rearrange("(p c) d -> p c d", p=P)).then_inc(sKW, 16)
            sp.dma_start(out=wv_sb, in_=wv.rearrange("(p c) d -> p c d", p=P)).then_inc(sVW, 16)
            for qi in range(4):
                if qi > 0:
                    sp.dma_start(out=xt_sb[:, 2 * qi:2 * qi + 2, :],
                                 in_=xr[:, 2 * qi:2 * qi + 2, :]).then_inc(sXT[qi], 16)
                sp.dma_start(out=zt_sb[:, 2 * qi:2 * qi + 2, :],
                             in_=zr[:, 2 * qi:2 * qi + 2, :]).then_inc(sZT[qi], 16)
            sp.dma_start(out=w0_sb, in_=w0[:, :]).then_inc(sW0, 16)
            w = WaitTracker(sp)
            for sc in range(SC):
                for sb in range(4):
                    row = sc * 512 + sb * P
                    di = sc * 4 + sb
                    w.need(sDVE, dve_ob(sc * 8 + 2 * sb + 1))
                    sp.dma_start(
                        out=out[row:row + P, :],
                        in_=ob_sb[:, di % NOB, :],
                    ).then_inc(sOB[di % NOB], 16)
            for j in range(NOB):
                nwrites = (SC * 4 + NOB - 1 - j) // NOB
                sp.wait_ge(sOB[j], 16 * nwrites)
            if os.environ.get("KDBG"):
                sDBG = nc.alloc_semaphore("sDBG")
                d_qP = nc.declare_dram_parameter("d_qP", [P, SC * 1024], mybir.dt.bfloat16, isOutput=True)
                d_kT = nc.declare_dram_parameter("d_kT", [P, S], mybir.dt.bfloat16, isOutput=True)
                d_v0 = nc.declare_dram_parameter("d_v0", [P, TB * 65], mybir.dt.bfloat16, isOutput=True)
                d_v1 = nc.declare_dram_parameter("d_v1", [P, TB * 65], mybir.dt.bfloat16, isOutput=True)
                d_ex = nc.declare_dram_parameter("d_ex", [P, NEX * 1024], mybir.dt.bfloat16, isOutput=True)
                d_rr = nc.declare_dram_parameter("d_rr", [1, 2 * 512], mybir.dt.float32, isOutput=True)
                d_oT = nc.declare_dram_parameter("d_oT", [P, 2 * 512], mybir.dt.bfloat16, isOutput=True)
                sp.dma_start(out=d_qP[:, :], in_=qP_sb).then_inc(sDBG, 16)
                sp.dma_start(out=d_kT[:, :], in_=kT_sb).then_inc(sDBG, 16)
                sp.dma_start(out=d_v0[:, :], in_=v0_sb).then_inc(sDBG, 16)
                sp.dma_start(out=d_v1[:, :], in_=v1_sb).then_inc(sDBG, 16)
                sp.dma_start(out=d_ex[:, :], in_=ex_sb).then_inc(sDBG, 16)
                sp.dma_start(out=d_rr[:, :], in_=rr_sb.bitcast(F32)).then_inc(sDBG, 16)
                sp.dma_start(out=d_oT[:, :], in_=oT_sb).then_inc(sDBG, 16)
                sp.wait_ge(sDBG, 16 * 7)

        @block.gpsimd
        def _(gp):
            gp.wait_ge(sGP, 1)
            from concourse.masks import make_identity
            make_identity(nc, ident, nomemset=True)
            nc.gpsimd.engine_nop().then_inc(sGP, 1)

        @block.tensor
        def _(pe):
            w = WaitTracker(pe)
            # Q projection: accumulate over ec into qa banks, per s-chunk.
            for ec in range(EC):
                w.need(sQW, 32)
                w.need(sXT[ec // 2], 16)
                for sc in range(SC):
                    i = nc.tensor.matmul(
                        (qa0 if sc < 2 else qa1)[:, (sc % 2) * 512:(sc % 2) * 512 + 512],
                        lhsT=wq_sb[:, ec, :],
                        rhs=xt_sb[:, ec, sc * 512:(sc + 1) * 512],
                        start=(ec == 0), stop=(ec == EC - 1),
                        skip_group_check=True,
                    )
                    if ec == EC - 1:
                        inc("PE", i, sPE, pe_q(sc))
            # K projection (no bias; constant shift cancels in softmax).
            for ec in range(EC):
                w.need(sKW, 16)
                w.need(sZT[ec // 2], 16)
                for sc in range(SC):
                    if ec == 0:
                        w.need(sDVE, dve_q(sc))
                    i = nc.tensor.matmul(
                        (qa0 if sc < 2 else qa1)[:, (sc % 2) * 512:(sc % 2) * 512 + 512],
                        lhsT=wk_sb[:, ec, :],
                        rhs=zt_sb[:, ec, sc * 512:(sc + 1) * 512],
                        start=(ec == 0), stop=(ec == EC - 1),
                        skip_group_check=True,
                    )
                    if ec == EC - 1:
                        inc("PE", i, sPE, pe_k(sc))
            # V projection as vT [dd, t] into qa banks, then transposes.
            for ec in range(EC):
                w.need(sVW, 16)
                w.need(sZT[ec // 2], 16)
                for sc in range(SC):
                    if ec == 0:
                        w.need(sDVE, dve_k(sc))
                    i = nc.tensor.matmul(
                        (qa0 if sc < 2 else qa1)[:, (sc % 2) * 512:(sc % 2) * 512 + 512],
                        lhsT=wv_sb[:, ec, :],
                        rhs=zt_sb[:, ec, sc * 512:(sc + 1) * 512],
                        start=(ec == 0), stop=(ec == EC - 1),
                        skip_group_check=True,
                    )
                    if ec == EC - 1:
                        inc("PE", i, sPE, pe_vT(sc))
            w.need(sGP, 2)
            for tb in range(TB):
                tgt = (bcp if tb % 2 == 0 else pjp)[0:P, 0:64].bitcast(BF16)
                w.need(sDVE, dve_vT(tb // 4))
                if tb >= 2:
                    w.need(sDVE, dve_vcopy(tb - 2))
                i = nc.tensor.transpose(tgt, vT_sb[:, tb * P:(tb + 1) * P], ident)
                inc("PE", i, sPE, pe_tp(tb))
            # Attention + projection, software-pipelined.
            for ent in ATTN_ORD:
                kind = ent[0]
                if kind == "scores":
                    _, sc, tb = ent
                    g = sc * TB + tb
                    qa = qa0 if tb % 2 == 0 else qa1
                    w.need(sDVE, dve_q(sc))
                    w.need(sDVE, dve_k(tb // 4))
                    # qa bank pair was last read by the vT copies of the
                    # two projection chunks it held
                    w.need(sDVE, dve_vT(1 if tb % 2 == 0 else 3))
                    if g >= 2:
                        w.need(sACT, g - 1)
                    # one shared kT stationary; two 512-wide streams (PSUM
                    # matmul output is limited to one bank)
                    nc.tensor.matmul(
                        qa[:, 0:512],
                        lhsT=kT_sb[:, tb * P:(tb + 1) * P],
                        rhs=qP_sb[:, sc, 0:512],
                        start=True, stop=True,
                    )
                    i = nc.tensor.matmul(
                        qa[:, 512:1024],
                        lhsT=kT_sb[:, tb * P:(tb + 1) * P],
                        rhs=qP_sb[:, sc, 512:1024],
                        start=True, stop=True,
                    )
                    inc("PE", i, sPE, pe_scores(sc, tb))
                elif kind == "av":
                    _, sc, tb = ent
                    g = sc * TB + tb
                    if tb == 0 and sc > 0:
                        w.need(sDVE, dve_ecp(sc - 1, 1))
                    w.need(sDVE, dve_vcopy(tb))
                    w.need(sACT, act_exp(sc, tb))
                    slot = g % NEX
                    nc.tensor.matmul(
                        av0[0:65, :],
                        lhsT=v0_sb[:, tb, :],
                        rhs=ex_sb[:, slot, 0:512],
                        start=(tb == 0), stop=(tb == TB - 1),
                        skip_group_check=True,
                    )
                    i = nc.tensor.matmul(
                        av1[0:65, :],
                        lhsT=v1_sb[:, tb, :],
                        rhs=ex_sb[:, slot, 512:1024],
                        start=(tb == 0), stop=(tb == TB - 1),
                        skip_group_check=True,
                    )
                    inc("PE", i, sPE, pe_av(sc, tb))
                elif kind == "bcast":
                    _, sc, h = ent
                    w.need(sDVE, dve_rcp(sc, h))
                    if h == 1:
                        w.need(sDVE, dve_bcs(sc, 0))
                    elif sc > 0:
                        w.need(sDVE, dve_ob((sc - 1) * 8 + 7))
                    i = nc.tensor.matmul(
                        bcp[0:64, :],
                        lhsT=ones_row[0:1, :],
                        rhs=rr_sb[0:1, h, :],
                        start=True, stop=True,
                    )
                    inc("PE", i, sPE, pe_bcast(sc, h))
                else:
                    _, sc, sb, oc = ent
                    gi = sc * 8 + sb * 2 + oc
                    bank = pjp if gi % 2 == 0 else bcp
                    w.need(sW0, 16)
                    w.need(sDVE, dve_mult(sc, 1))
                    if gi >= 2:
                        w.need(sDVE, dve_ob(gi - 2))
                    i = nc.tensor.matmul(
                        bank[:, :],
                        lhsT=oT_sb[:, sc % 2, sb * P:(sb + 1) * P],
                        rhs=w0_sb[:, oc * 512:(oc + 1) * 512],
                        start=True, stop=True,
                    )
                    inc("PE", i, sPE, pe_proj(sc, sb, oc))

        @block.scalar
        def _(act):
            w = WaitTracker(act)
            for sc in range(SC):
                for tb in range(TB):
                    g = sc * TB + tb
                    w.need(sPE, pe_scores(sc, tb))
                    if g >= NEX:
                        gp_sc, gp_tb = divmod(g - NEX, TB)
                        w.need(sPE, pe_av(gp_sc, gp_tb))
                    slot = g % NEX
                    qa = qa0 if tb % 2 == 0 else qa1
                    i = nc.scalar.activation(
                        ex_sb[:, slot, :], qa[:, :], Exp, scale=0.125)
                    inc("ACT", i, sACT, act_exp(sc, tb))

        @block.vector
        def _(dve):
            w = WaitTracker(dve)
            # zero the q pads once; later ticks imply completion (in-order)
            nc.vector.memset(ident, 0.0).then_inc(sGP, 1)
            nc.vector.memset(qP_sb[64:P, :, 0:512], 0.0)
            nc.vector.memset(qP_sb[0:64, :, 512:1024], 0.0)
            nc.vector.memset(ones_row.bitcast(F32), 1.0)
            nc.vector.memset(v0_sb[:, :, 64:65].bitcast(F32), 1.0)
            nc.vector.memset(v1_sb[:, :, 64:65].bitcast(F32), 1.0)
            # q: bias-add + cast into padded layout
            for sc in range(SC):
                w.need(sQW, 32)
                w.need(sPE, pe_q(sc))
                qa = (qa0 if sc < 2 else qa1)[:, (sc % 2) * 512:(sc % 2) * 512 + 512]
                nc.vector.tensor_scalar_add(
                    out=qP_sb[0:64, sc, 0:512],
                    in0=qa[0:64, :],
                    scalar1=bq_sb[0:64, 0:1],
                )
                i = nc.vector.tensor_scalar_add(
                    out=qP_sb[64:P, sc, 512:1024],
                    in0=qa[64:P, :],
                    scalar1=bq_sb[64:P, 0:1],
                )
                inc("DVE", i, sDVE, dve_q(sc))
            # k: plain cast copy
            for sc in range(SC):
                w.need(sPE, pe_k(sc))
                qa = (qa0 if sc < 2 else qa1)[:, (sc % 2) * 512:(sc % 2) * 512 + 512]
                i = nc.vector.tensor_copy(kT_sb[:, sc * 512:(sc + 1) * 512], qa)
                inc("DVE", i, sDVE, dve_k(sc))
            # vT: cast copy out of qa banks
            for sc in range(SC):
                w.need(sPE, pe_vT(sc))
                qa = (qa0 if sc < 2 else qa1)[:, (sc % 2) * 512:(sc % 2) * 512 + 512]
                i = nc.vector.tensor_copy(vT_sb[:, sc * 512:(sc + 1) * 512], qa)
                inc("DVE", i, sDVE, dve_vT(sc))
            # v: split transposed [t, dd] blocks into per-head [t, 64] slots
            for tb in range(TB):
                src = (bcp if tb % 2 == 0 else pjp)[0:P, 0:64].bitcast(BF16)
                w.need(sPE, pe_tp(tb))
                nc.vector.tensor_copy(v0_sb[:, tb, 0:64], src[:, 0:64])
                i = nc.vector.tensor_copy(v1_sb[:, tb, 0:64], src[:, 64:128])
                inc("DVE", i, sDVE, dve_vcopy(tb))
            # attention normalization + output staging
            for sc in range(SC):
                for h, av in ((0, av0), (1, av1)):
                    w.need(sPE, pe_av(sc, TB - 1))
                    i = nc.vector.tensor_copy(E_sb[0:65, h, :], av[0:65, :])
                    inc("DVE", i, sDVE, dve_ecp(sc, h))
                for h in range(2):
                    w.need(sDVE, dve_ecp(sc, h))
                    i = nc.vector.reciprocal(rr_sb[0:1, h, :], E_sb[64:65, h, :])
                    inc("DVE", i, sDVE, dve_rcp(sc, h))
                for h in range(2):
                    w.need(sPE, pe_bcast(sc, h))
                    if h == 1:
                        w.need(sDVE, dve_mult(sc, 0))
                    elif sc > 0:
                        w.need(sDVE, dve_mult(sc - 1, 1))
                    i = nc.vector.tensor_copy(bcs_sb, bcp[0:64, :])
                    inc("DVE", i, sDVE, dve_bcs(sc, h))
                    w.need(sDVE, dve_bcs(sc, h))
                    i = nc.vector.tensor_mul(
                        oT_sb[h * 64:(h + 1) * 64, sc % 2, :],
                        E_sb[0:64, h, :], bcs_sb)
                    inc("DVE", i, sDVE, dve_mult(sc, h))
                for j in range(8):
                    sb, oc = divmod(j, 2)
                    gi = sc * 8 + j
                    di = sc * 4 + sb
                    bank = pjp if gi % 2 == 0 else bcp
                    w.need(sPE, pe_proj(sc, sb, oc))
                    if di >= NOB:
                        w.need(sOB[di % NOB], 16 * (di // NOB))
                    i = nc.vector.tensor_copy(
                        ob_sb[:, di % NOB, oc * 512:(oc + 1) * 512], bank[:, :])
                    inc("DVE", i, sDVE, dve_ob(gi))

    _lp.close()
    return nc


def _get_nc():
    if "nc" not in _built:
        _built["nc"] = _build_bass()
    return _built["nc"]


def _make_in_maps(x, z, Wq, bq, Wk, Wv, W0):
    import concourse.mybir as mybir
    BF = mybir.dt.np(mybir.dt.bfloat16)
    xT = np.ascontiguousarray(x.T).astype(BF)
    zT = np.ascontiguousarray(z.T).astype(BF)
    in_maps = []
    for c in range(NCORES):
        h0, h1 = 2 * c, 2 * c + 1
        in_maps.append({
            "xT": xT,
            "zT": zT,
            "wq": np.ascontiguousarray(np.concatenate([Wq[h0], Wq[h1]], axis=1)).astype(BF),
            "wk": np.ascontiguousarray(np.concatenate([Wk[h0], Wk[h1]], axis=1)).astype(BF),
            "wv": np.ascontiguousarray(np.concatenate([Wv[h0], Wv[h1]], axis=1)).astype(BF),
            "bq": np.ascontiguousarray(np.concatenate([bq[h0], bq[h1]]).reshape(DD, 1), np.float32),
            "w0": np.ascontiguousarray(W0[c * DD:(c + 1) * DD, :]).astype(BF),
        })
    return in_maps


def _numpy_reference(x, z, mask, Wq, bq, Wk, bk, Wv, bv, W0, b0):
    # general-mask fallback (not the benchmarked path; harness mask is all-ones)
    x = x.astype(np.float64); z = z.astype(np.float64)
    q = np.einsum("se,hed->hsd", x, Wq) + bq[:, None, :]
    k = np.einsum("te,hed->htd", z, Wk) + bk[:, None, :]
    v = np.einsum("te,hem->htm", z, Wv) + bv[:, None, :]
    s = np.einsum("hsd,htd->hst", q, k) / np.sqrt(np.float64(D))
    s = np.where(mask[None, :, :] == 0, -np.inf, s)
    s = s - s.max(axis=-1, keepdims=True)
    e = np.exp(s)
    a = e / e.sum(axis=-1, keepdims=True)
    o = np.einsum("hst,htm->hsm", a, v)
    o = np.transpose(o, (1, 0, 2)).reshape(S, H * MD)
    return (o @ W0 + b0).astype(np.float32)


def kernel(x, z, mask, Wq, bq, Wk, bk, Wv, bv, W0, b0):
    global LAST_EXEC_TIME_NS, LAST_RESULTS
    arrs = {k: np.asarray(v) for k, v in dict(
        x=x, z=z, mask=mask, Wq=Wq, bq=bq, Wk=Wk, bk=bk, Wv=Wv, bv=bv,
        W0=W0, b0=b0).items()}
    if not bool((arrs["mask"] != 0).all()):
        return _numpy_reference(**arrs)

    from concourse.bass_utils import run_bass_kernel_spmd

    nc = _get_nc()
    in_maps = _make_in_maps(
        arrs["x"], arrs["z"], arrs["Wq"], arrs["bq"], arrs["Wk"],
        arrs["Wv"], arrs["W0"])
    trace = bool(os.environ.get("KERNEL_TRACE"))
    kw = {}
    td = os.environ.get("KERNEL_TRACE_DIR")
    if td:
        os.makedirs(td, exist_ok=True)
        kw["tmpdir"] = td
    res = run_bass_kernel_spmd(
        nc, in_maps, core_ids=list(range(NCORES)), trace=trace, **kw
    )
    LAST_EXEC_TIME_NS = res.exec_time_ns
    LAST_RESULTS = res
    acc = np.zeros((S, O), dtype=np.float32)
    for rm in res.results:
        acc += rm["out"]
    # bv is not applied on-device: sum_t softmax * bv == bv, so it folds
    # into the final bias through W0.
    b0p = (arrs["b0"].astype(np.float64)
           + arrs["bv"].reshape(-1).astype(np.float64) @ arrs["W0"].astype(np.float64))
    acc += b0p.astype(np.float32)[None, :]
    return acc


# revision 17
# speedup vs baseline: 1.4407x; 1.0123x over previous
"""Multi-head attention (16 heads, S=2048, E=1024, D=M=64, O=1024) on 8 trn2
NeuronCores, head-sharded: 2 heads per core, partial output summed on host.

v2: bf16 matmul datapath (inputs host-cast), single-matmul scores via
zero-padded qT, direct [t,m] V projection (no transposes), fast reciprocal,
reordered DMA with split weight semaphores. bk is dropped (constant shift
along the softmax axis), bv is folded into b0 on host.

Self-contained: hardcodes all shapes; builds a Bass program and runs it via
concourse.bass_utils.run_bass_kernel_spmd on cores 0-7.
"""

import os
import sys

import numpy as np

# hardcoded problem shapes
H, E, D, MD, O, S = 16, 1024, 64, 64, 1024, 2048
NCORES = 8
HPC = H // NCORES          # heads per core = 2
DD = HPC * D               # packed head dim rows = 128
P = 128

# filled by the last device run (for test harness)
LAST_EXEC_TIME_NS = None
LAST_RESULTS = None

_REPO = "/opt/trn_rl_repo"
if _REPO not in sys.path:
    sys.path.insert(0, _REPO)

_built = {}


def _build_bass():
    import concourse.bass as bass
    import concourse.mybir as mybir

    F32 = mybir.dt.float32
    F32R = mybir.dt.float32r
    BF16 = mybir.dt.bfloat16
    Exp = mybir.ActivationFunctionType.Exp

    nc = bass.Bass()
    import contextlib
    _lp = contextlib.ExitStack()
    _lp.enter_context(nc.allow_low_precision(
        reason="bf16 datapath is within the 2e-2 harness tolerance"))

    xT = nc.declare_dram_parameter("xT", [E, S], BF16, isOutput=False)
    zT = nc.declare_dram_parameter("zT", [E, S], BF16, isOutput=False)
    wq = nc.declare_dram_parameter("wq", [E, DD], BF16, isOutput=False)
    wk = nc.declare_dram_parameter("wk", [E, DD], BF16, isOutput=False)
    wv = nc.declare_dram_parameter("wv", [E, DD], BF16, isOutput=False)
    bq = nc.declare_dram_parameter("bq", [DD, 1], F32, isOutput=False)
    w0 = nc.declare_dram_parameter("w0", [DD, O], BF16, isOutput=False)
    out = nc.declare_dram_parameter("out", [S, O], F32, isOutput=True)

    EC = E // P               # 8 e-chunks
    SC = S // 512             # 4 s-chunks of 512
    TB = S // P               # 16 t-blocks
    NEX = 4                   # exp sbuf slots
    NOB = 4                   # output staging slots of [P, 1024]

    # ---- static SBUF allocation --------------------------------------
    xt_sb = nc.alloc_sbuf_tensor("xt_sb", [P, EC, S], BF16).ap()
    zt_sb = nc.alloc_sbuf_tensor("zt_sb", [P, EC, S], BF16).ap()
    # padded q: cols 0:512 head0 (rows 64:128 zero), 512:1024 head1 (rows 0:64 zero)
    qP_sb = nc.alloc_sbuf_tensor("qP_sb", [P, SC, 1024], BF16).ap()
    kT_sb = nc.alloc_sbuf_tensor("kT_sb", [P, S], BF16).ap()
    wq_sb = nc.alloc_sbuf_tensor("wq_sb", [P, EC, DD], BF16).ap()
    wk_sb = nc.alloc_sbuf_tensor("wk_sb", [P, EC, DD], BF16).ap()
    wv_sb = nc.alloc_sbuf_tensor("wv_sb", [P, EC, DD], BF16).ap()
    w0_sb = nc.alloc_sbuf_tensor("w0_sb", [P, O], BF16).ap()
    bq_sb = nc.alloc_sbuf_tensor("bq_sb", [P, 1], F32).ap()
    ones_row = nc.alloc_sbuf_tensor("ones_row", [1, 64], F32R).ap()
    vT_sb = nc.alloc_sbuf_tensor("vT_sb", [P, S], BF16).ap()
    ident = nc.alloc_sbuf_tensor("ident", [P, P], BF16).ap()
    v0_sb = nc.alloc_sbuf_tensor("v0_sb", [P, TB, 65], F32R).ap()
    v1_sb = nc.alloc_sbuf_tensor("v1_sb", [P, TB, 65], F32R).ap()
    ex_sb = nc.alloc_sbuf_tensor("ex_sb", [P, NEX, 1024], F32R).ap()
    E_sb = nc.alloc_sbuf_tensor("E_sb", [P, 2, 512], F32).ap()
    lnt_sb = nc.alloc_sbuf_tensor("lnt_sb", [1, 2, 512], F32).ap()
    scr_sb = nc.alloc_sbuf_tensor("scr_sb", [1, 2], F32).ap()
    rr_sb = nc.alloc_sbuf_tensor("rr_sb", [1, 2, 512], F32R).ap()
    bcs_sb = nc.alloc_sbuf_tensor("bcs_sb", [64, 512], F32).ap()
    oT_sb = nc.alloc_sbuf_tensor("oT_sb", [P, 2, 512], BF16).ap()
    ob_sb = nc.alloc_sbuf_tensor("ob_sb", [P, NOB, 1024], F32).ap()

    # ---- static PSUM banks -------------------------------------------
    qa0 = nc.alloc_psum_tensor("qa0", [P, 1024], F32).ap()   # banks 0-1
    qa1 = nc.alloc_psum_tensor("qa1", [P, 1024], F32).ap()   # banks 2-3
    av0 = nc.alloc_psum_tensor("av0", [P, 512], F32).ap()    # bank 4
    av1 = nc.alloc_psum_tensor("av1", [P, 512], F32).ap()    # bank 5
    bcp = nc.alloc_psum_tensor("bcp", [P, 512], F32).ap()    # bank 6
    pjp = nc.alloc_psum_tensor("pjp", [P, 512], F32).ap()    # bank 7

    # ---- semaphores ---------------------------------------------------
    sQW = nc.alloc_semaphore("sQW")                          # wq+bq: 32
    sKW = nc.alloc_semaphore("sKW")                          # wk: 16
    sVW = nc.alloc_semaphore("sVW")                          # wv: 16
    sW0 = nc.alloc_semaphore("sW0")
    sXT = [nc.alloc_semaphore(f"sXT{c}") for c in range(4)]
    sZT = [nc.alloc_semaphore(f"sZT{c}") for c in range(4)]
    sOB = [nc.alloc_semaphore(f"sOB{j}") for j in range(NOB)]
    sGP = nc.alloc_semaphore("sGP")
    sLN = nc.alloc_semaphore("sLN")
    sPE = nc.alloc_semaphore("sPE")
    sACT = nc.alloc_semaphore("sACT")
    sDVE = nc.alloc_semaphore("sDVE")

    # ---- closed-form tick schedules ----------------------------------
    # PE ticks: q sc (4), k sc (4), v tb (16), then attention entries.
    def pe_q(sc):
        return sc + 1

    def pe_k(sc):
        return 4 + sc + 1

    def pe_vT(sc):
        return 8 + sc + 1

    def pe_tp(tb):
        return 12 + tb + 1

    def pe_scores(sc, tb):
        return PE_TICK[("scores", sc, tb)]

    def pe_av(sc, tb):
        return PE_TICK[("av", sc, tb)]

    def pe_bcast(sc, h):
        return PE_TICK[("bcast", sc, h)]

    def pe_proj(sc, sb, oc):
        return PE_TICK[("proj", sc, sb, oc)]

    # ACT: one tick per exp
    def act_exp(sc, tb):
        return sc * TB + tb + 1

    # DVE ticks: q sc (4), k sc (4), v tb (16), then per sc:
    # recip x2, [bcs, mult] x2, ob x8 -> 14 per sc.
    def dve_q(sc):
        return sc + 1

    def dve_k(sc):
        return 4 + sc + 1

    def dve_vT(sc):
        return 8 + sc + 1

    def dve_vcopy(tb):
        return 12 + tb + 1

    def dve_ecp(sc, h):
        return 28 + sc * 16 + h + 1

    def dve_rcp(sc, h):
        return 28 + sc * 16 + 2 + h + 1

    def dve_bcs(sc, h):
        return 28 + sc * 16 + 4 + 2 * h + 1

    def dve_mult(sc, h):
        return 28 + sc * 16 + 4 + 2 * h + 2

    def dve_ob(gi):
        sc, j = divmod(gi, 8)
        return 28 + sc * 16 + 8 + j + 1

    # software-pipelined PE attention order: scores run 2 iterations
    # ahead of AV.
    ATTN_ORD = [("scores", 0, 0), ("scores", 0, 1)]
    for sc_ in range(SC):
        for tb_ in range(TB):
            ATTN_ORD.append(("av", sc_, tb_))
            gn = sc_ * TB + tb_ + 2
            if gn < SC * TB:
                ATTN_ORD.append(("scores", gn // TB, gn % TB))
            # previous chunk's normalization-dependent PE work, deferred
            # deep enough that the ~4us reciprocals run off the critical path
            if sc_ > 0:
                pv = sc_ - 1
                for j_ in {6: [-1], 9: [-2], 10: [0], 11: [1, 2], 12: [3, 4],
                           13: [5, 6], 14: [7]}.get(tb_, []):
                    if j_ == -1:
                        ATTN_ORD.append(("bcast", pv, 0))
                    elif j_ == -2:
                        ATTN_ORD.append(("bcast", pv, 1))
                    else:
                        ATTN_ORD.append(("proj", pv, j_ // 2, j_ % 2))
    for h_ in range(2):
        ATTN_ORD.append(("bcast", SC - 1, h_))
    for sb_ in range(4):
        for oc_ in range(2):
            ATTN_ORD.append(("proj", SC - 1, sb_, oc_))
    PE_TICK = {e: 28 + i + 1 for i, e in enumerate(ATTN_ORD)}

    counts = {"PE": 0, "ACT": 0, "DVE": 0}

    def inc(eng, instr, sem, expect):
        instr.then_inc(sem, 1)
        counts[eng] += 1
        assert counts[eng] == expect, (eng, counts[eng], expect)

    class WaitTracker:
        def __init__(self, eng):
            self.eng = eng
            self.seen = {}

        def need(self, sem, val):
            if val <= 0:
                return
            key = sem.name
            if self.seen.get(key, -1) >= val:
                return
            self.seen[key] = val
            self.eng.wait_ge(sem, val)

    with nc.Block() as block:

        @block.sync
        def _(sp):
            sp.dma_start(out=wq_sb, in_=wq.rearrange("(p c) d -> p c d", p=P)).then_inc(sQW, 16)
            sp.dma_start(out=bq_sb, in_=bq[:, :]).then_inc(sQW, 16)
            xr = xT.rearrange("(p c) d -> p c d", p=P)
            for qi in range(4):
                sp.dma_start(out=xt_sb[:, 2 * qi:2 * qi + 2, :],
                             in_=xr[:, 2 * qi:2 * qi + 2, :]).then_inc(sXT[qi], 16)
            sp.dma_start(out=w0_sb, in_=w0[:, :]).then_inc(sW0, 16)
            w = WaitTracker(sp)
            for sc in range(SC):
                for sb in range(4):
                    di = sc * 4 + sb
                    if di % 2 != 0:
                        continue
                    row = sc * 512 + sb * P
                    w.need(sDVE, dve_ob(sc * 8 + 2 * sb + 1))
                    sp.dma_start(
                        out=out[row:row + P, :],
                        in_=ob_sb[:, di % NOB, :],
                    ).then_inc(sOB[di % NOB], 16)
            for j in range(NOB):
                nwrites = (SC * 4 + NOB - 1 - j) // NOB
                sp.wait_ge(sOB[j], 16 * nwrites)
            if os.environ.get("KDBG"):
                sDBG = nc.alloc_semaphore("sDBG")
                d_qP = nc.declare_dram_parameter("d_qP", [P, SC * 1024], mybir.dt.bfloat16, isOutput=True)
                d_kT = nc.declare_dram_parameter("d_kT", [P, S], mybir.dt.bfloat16, isOutput=True)
                d_v0 = nc.declare_dram_parameter("d_v0", [P, TB * 65], mybir.dt.bfloat16, isOutput=True)
                d_v1 = nc.declare_dram_parameter("d_v1", [P, TB * 65], mybir.dt.bfloat16, isOutput=True)
                d_ex = nc.declare_dram_parameter("d_ex", [P, NEX * 1024], mybir.dt.bfloat16, isOutput=True)
                d_rr = nc.declare_dram_parameter("d_rr", [1, 2 * 512], mybir.dt.float32, isOutput=True)
                d_oT = nc.declare_dram_parameter("d_oT", [P, 2 * 512], mybir.dt.bfloat16, isOutput=True)
                sp.dma_start(out=d_qP[:, :], in_=qP_sb).then_inc(sDBG, 16)
                sp.dma_start(out=d_kT[:, :], in_=kT_sb).then_inc(sDBG, 16)
                sp.dma_start(out=d_v0[:, :], in_=v0_sb).then_inc(sDBG, 16)
                sp.dma_start(out=d_v1[:, :], in_=v1_sb).then_inc(sDBG, 16)
                sp.dma_start(out=d_ex[:, :], in_=ex_sb).then_inc(sDBG, 16)
                sp.dma_start(out=d_rr[:, :], in_=rr_sb.bitcast(F32)).then_inc(sDBG, 16)
                sp.dma_start(out=d_oT[:, :], in_=oT_sb).then_inc(sDBG, 16)
                sp.wait_ge(sDBG, 16 * 7)

        @block.gpsimd
        def _(gp):
            gp.wait_ge(sGP, 1)
            from concourse.masks import make_identity
            make_identity(nc, ident, nomemset=True)
            nc.gpsimd.engine_nop().then_inc(sGP, 1)
            gw = WaitTracker(gp)
            for sc in range(SC):
                for sb in range(4):
                    di = sc * 4 + sb
                    if di % 2 != 1:
                        continue
                    row = sc * 512 + sb * P
                    gw.need(sDVE, dve_ob(sc * 8 + 2 * sb + 1))
                    gp.dma_start(
                        out=out[row:row + P, :],
                        in_=ob_sb[:, di % NOB, :],
                    ).then_inc(sOB[di % NOB], 16)

        @block.tensor
        def _(pe):
            w = WaitTracker(pe)
            # Q projection: accumulate over ec into qa banks, per s-chunk.
            for ec in range(EC):
                w.need(sQW, 32)
                w.need(sXT[ec // 2], 16)
                for sc in range(SC):
                    i = nc.tensor.matmul(
                        (qa0 if sc < 2 else qa1)[:, (sc % 2) * 512:(sc % 2) * 512 + 512],
                        lhsT=wq_sb[:, ec, :],
                        rhs=xt_sb[:, ec, sc * 512:(sc + 1) * 512],
                        start=(ec == 0), stop=(ec == EC - 1),
                        skip_group_check=True,
                    )
                    if ec == EC - 1:
                        inc("PE", i, sPE, pe_q(sc))
            # K projection (no bias; constant shift cancels in softmax).
            for ec in range(EC):
                w.need(sKW, 16)
                w.need(sZT[ec // 2], 16)
                for sc in range(SC):
                    if ec == 0:
                        w.need(sDVE, dve_q(sc))
                    i = nc.tensor.matmul(
                        (qa0 if sc < 2 else qa1)[:, (sc % 2) * 512:(sc % 2) * 512 + 512],
                        lhsT=wk_sb[:, ec, :],
                        rhs=zt_sb[:, ec, sc * 512:(sc + 1) * 512],
                        start=(ec == 0), stop=(ec == EC - 1),
                        skip_group_check=True,
                    )
                    if ec == EC - 1:
                        inc("PE", i, sPE, pe_k(sc))
            # V projection as vT [dd, t] into qa banks, then transposes.
            for ec in range(EC):
                w.need(sVW, 16)
                w.need(sZT[ec // 2], 16)
                for sc in range(SC):
                    if ec == 0:
                        w.need(sDVE, dve_k(sc))
                    i = nc.tensor.matmul(
                        (qa0 if sc < 2 else qa1)[:, (sc % 2) * 512:(sc % 2) * 512 + 512],
                        lhsT=wv_sb[:, ec, :],
                        rhs=zt_sb[:, ec, sc * 512:(sc + 1) * 512],
                        start=(ec == 0), stop=(ec == EC - 1),
                        skip_group_check=True,
                    )
                    if ec == EC - 1:
                        inc("PE", i, sPE, pe_vT(sc))
            w.need(sGP, 2)
            for tb in range(TB):
                tgt = (bcp if tb % 2 == 0 else pjp)[0:P, 0:64].bitcast(BF16)
                w.need(sDVE, dve_vT(tb // 4))
                if tb >= 2:
                    w.need(sDVE, dve_vcopy(tb - 2))
                i = nc.tensor.transpose(tgt, vT_sb[:, tb * P:(tb + 1) * P], ident)
                inc("PE", i, sPE, pe_tp(tb))
            # Attention + projection, software-pipelined.
            for ent in ATTN_ORD:
                kind = ent[0]
                if kind == "scores":
                    _, sc, tb = ent
                    g = sc * TB + tb
                    qa = qa0 if tb % 2 == 0 else qa1
                    w.need(sDVE, dve_q(sc))
                    w.need(sDVE, dve_k(tb // 4))
                    # qa bank pair was last read by the vT copies of the
                    # two projection chunks it held
                    w.need(sDVE, dve_vT(1 if tb % 2 == 0 else 3))
                    if g >= 2:
                        w.need(sACT, g - 1)
                    # one shared kT stationary; two 512-wide streams (PSUM
                    # matmul output is limited to one bank)
                    nc.tensor.matmul(
                        qa[:, 0:512],
                        lhsT=kT_sb[:, tb * P:(tb + 1) * P],
                        rhs=qP_sb[:, sc, 0:512],
                        start=True, stop=True,
                    )
                    i = nc.tensor.matmul(
                        qa[:, 512:1024],
                        lhsT=kT_sb[:, tb * P:(tb + 1) * P],
                        rhs=qP_sb[:, sc, 512:1024],
                        start=True, stop=True,
                    )
                    inc("PE", i, sPE, pe_scores(sc, tb))
                elif kind == "av":
                    _, sc, tb = ent
                    g = sc * TB + tb
                    if tb == 0 and sc > 0:
                        w.need(sDVE, dve_ecp(sc - 1, 1))
                    w.need(sDVE, dve_vcopy(tb))
                    w.need(sACT, act_exp(sc, tb))
                    slot = g % NEX
                    nc.tensor.matmul(
                        av0[0:65, :],
                        lhsT=v0_sb[:, tb, :],
                        rhs=ex_sb[:, slot, 0:512],
                        start=(tb == 0), stop=(tb == TB - 1),
                        skip_group_check=True,
                    )
                    i = nc.tensor.matmul(
                        av1[0:65, :],
                        lhsT=v1_sb[:, tb, :],
                        rhs=ex_sb[:, slot, 512:1024],
                        start=(tb == 0), stop=(tb == TB - 1),
                        skip_group_check=True,
                    )
                    inc("PE", i, sPE, pe_av(sc, tb))
                elif kind == "bcast":
                    _, sc, h = ent
                    if sc == SC - 1:
                        w.need(sACT, SC * TB + h + 1)
                    else:
                        w.need(sDVE, dve_rcp(sc, h))
                    if h == 1:
                        w.need(sDVE, dve_bcs(sc, 0))
                    elif sc > 0:
                        w.need(sDVE, dve_ob((sc - 1) * 8 + 7))
                    i = nc.tensor.matmul(
                        bcp[0:64, :],
                        lhsT=ones_row[0:1, :],
                        rhs=rr_sb[0:1, h, :],
                        start=True, stop=True,
                    )
                    inc("PE", i, sPE, pe_bcast(sc, h))
                else:
                    _, sc, sb, oc = ent
                    gi = sc * 8 + sb * 2 + oc
                    bank = pjp if gi % 2 == 0 else bcp
                    w.need(sW0, 16)
                    w.need(sDVE, dve_mult(sc, 1))
                    if gi >= 2:
                        w.need(sDVE, dve_ob(gi - 2))
                    i = nc.tensor.matmul(
                        bank[:, :],
                        lhsT=oT_sb[:, sc % 2, sb * P:(sb + 1) * P],
                        rhs=w0_sb[:, oc * 512:(oc + 1) * 512],
                        start=True, stop=True,
                    )
                    inc("PE", i, sPE, pe_proj(sc, sb, oc))

        @block.scalar
        def _(act):
            w = WaitTracker(act)
            zr = zT.rearrange("(p c) d -> p c d", p=P)
            act.dma_start(out=wk_sb, in_=wk.rearrange("(p c) d -> p c d", p=P)).then_inc(sKW, 16)
            act.dma_start(out=wv_sb, in_=wv.rearrange("(p c) d -> p c d", p=P)).then_inc(sVW, 16)
            for qi in range(4):
                act.dma_start(out=zt_sb[:, 2 * qi:2 * qi + 2, :],
                              in_=zr[:, 2 * qi:2 * qi + 2, :]).then_inc(sZT[qi], 16)
            for sc in range(SC):
                for tb in range(TB):
                    g = sc * TB + tb
                    w.need(sPE, pe_scores(sc, tb))
                    if g >= NEX:
                        gp_sc, gp_tb = divmod(g - NEX, TB)
                        w.need(sPE, pe_av(gp_sc, gp_tb))
                    slot = g % NEX
                    qa = qa0 if tb % 2 == 0 else qa1
                    i = nc.scalar.activation(
                        ex_sb[:, slot, :], qa[:, :], Exp, scale=0.125)
                    inc("ACT", i, sACT, act_exp(sc, tb))
            # last chunk's reciprocal via Ln+Exp (ACT is idle by then), so the
            # tail does not pay the ~4us DVE reciprocal
            for h in range(2):
                w.need(sDVE, dve_ecp(SC - 1, h))
                nc.scalar.activation(lnt_sb[0:1, h, :], E_sb[64:65, h, :],
                                     mybir.ActivationFunctionType.Ln).then_inc(sLN, 1)
                w.need(sLN, h + 1)
                i = nc.scalar.activation(rr_sb[0:1, h, :], lnt_sb[0:1, h, :],
                                         Exp, scale=-1.0)
                inc("ACT", i, sACT, SC * TB + h + 1)

        @block.vector
        def _(dve):
            w = WaitTracker(dve)
            # zero the q pads once; later ticks imply completion (in-order)
            nc.vector.memset(ident, 0.0).then_inc(sGP, 1)
            nc.vector.memset(qP_sb[64:P, :, 0:512], 0.0)
            nc.vector.memset(qP_sb[0:64, :, 512:1024], 0.0)
            nc.vector.memset(ones_row.bitcast(F32), 1.0)
            nc.vector.memset(v0_sb[:, :, 64:65].bitcast(F32), 1.0)
            nc.vector.memset(v1_sb[:, :, 64:65].bitcast(F32), 1.0)
            # q: bias-add + cast into padded layout
            for sc in range(SC):
                w.need(sQW, 32)
                w.need(sPE, pe_q(sc))
                qa = (qa0 if sc < 2 else qa1)[:, (sc % 2) * 512:(sc % 2) * 512 + 512]
                nc.vector.tensor_scalar_add(
                    out=qP_sb[0:64, sc, 0:512],
                    in0=qa[0:64, :],
                    scalar1=bq_sb[0:64, 0:1],
                )
                i = nc.vector.tensor_scalar_add(
                    out=qP_sb[64:P, sc, 512:1024],
                    in0=qa[64:P, :],
                    scalar1=bq_sb[64:P, 0:1],
                )
                inc("DVE", i, sDVE, dve_q(sc))
            # k: plain cast copy
            for sc in range(SC):
                w.need(sPE, pe_k(sc))
                qa = (qa0 if sc < 2 else qa1)[:, (sc % 2) * 512:(sc % 2) * 512 + 512]
                i = nc.vector.tensor_copy(kT_sb[:, sc * 512:(sc + 1) * 512], qa)
                inc("DVE", i, sDVE, dve_k(sc))
            # vT: cast copy out of qa banks
            for sc in range(SC):
                w.need(sPE, pe_vT(sc))
                qa = (qa0 if sc < 2 else qa1)[:, (sc % 2) * 512:(sc % 2) * 512 + 512]
                i = nc.vector.tensor_copy(vT_sb[:, sc * 512:(sc + 1) * 512], qa)
                inc("DVE", i, sDVE, dve_vT(sc))
            # v: split transposed [t, dd] blocks into per-head [t, 64] slots
            for tb in range(TB):
                src = (bcp if tb % 2 == 0 else pjp)[0:P, 0:64].bitcast(BF16)
                w.need(sPE, pe_tp(tb))
                nc.vector.tensor_copy(v0_sb[:, tb, 0:64], src[:, 0:64])
                i = nc.vector.tensor_copy(v1_sb[:, tb, 0:64], src[:, 64:128])
                inc("DVE", i, sDVE, dve_vcopy(tb))
            # attention normalization + output staging
            for sc in range(SC):
                for h, av in ((0, av0), (1, av1)):
                    w.need(sPE, pe_av(sc, TB - 1))
                    i = nc.vector.tensor_copy(E_sb[0:65, h, :], av[0:65, :])
                    inc("DVE", i, sDVE, dve_ecp(sc, h))
                for h in range(2):
                    w.need(sDVE, dve_ecp(sc, h))
                    if sc == SC - 1:
                        i = nc.vector.memset(scr_sb[0:1, h:h + 1], 0.0)
                    else:
                        i = nc.vector.reciprocal(rr_sb[0:1, h, :], E_sb[64:65, h, :])
                    inc("DVE", i, sDVE, dve_rcp(sc, h))
                for h in range(2):
                    w.need(sPE, pe_bcast(sc, h))
                    if h == 1:
                        w.need(sDVE, dve_mult(sc, 0))
                    elif sc > 0:
                        w.need(sDVE, dve_mult(sc - 1, 1))
                    i = nc.vector.tensor_copy(bcs_sb, bcp[0:64, :])
                    inc("DVE", i, sDVE, dve_bcs(sc, h))
                    w.need(sDVE, dve_bcs(sc, h))
                    i = nc.vector.tensor_mul(
                        oT_sb[h * 64:(h + 1) * 64, sc % 2, :],
                        E_sb[0:64, h, :], bcs_sb)
                    inc("DVE", i, sDVE, dve_mult(sc, h))
                for j in range(8):
                    sb, oc = divmod(j, 2)
                    gi = sc * 8 + j
                    di = sc * 4 + sb
                    bank = pjp if gi % 2 == 0 else bcp
                    w.need(sPE, pe_proj(sc, sb, oc))
                    if di >= NOB:
                        w.need(sOB[di % NOB], 16 * (di // NOB))
                    i = nc.vector.tensor_copy(
                        ob_sb[:, di % NOB, oc * 512:(oc + 1) * 512], bank[:, :])
                    inc("DVE", i, sDVE, dve_ob(gi))

    _lp.close()
    return nc


def _get_nc():
    if "nc" not in _built:
        _built["nc"] = _build_bass()
    return _built["nc"]


def _make_in_maps(x, z, Wq, bq, Wk, Wv, W0):
    import concourse.mybir as mybir
    BF = mybir.dt.np(mybir.dt.bfloat16)
    xT = np.ascontiguousarray(x.T).astype(BF)
    zT = np.ascontiguousarray(z.T).astype(BF)
    in_maps = []
    for c in range(NCORES):
        h0, h1 = 2 * c, 2 * c + 1
        in_maps.append({
            "xT": xT,
            "zT": zT,
            "wq": np.ascontiguousarray(np.concatenate([Wq[h0], Wq[h1]], axis=1)).astype(BF),
            "wk": np.ascontiguousarray(np.concatenate([Wk[h0], Wk[h1]], axis=1)).astype(BF),
            "wv": np.ascontiguousarray(np.concatenate([Wv[h0], Wv[h1]], axis=1)).astype(BF),
            "bq": np.ascontiguousarray(np.concatenate([bq[h0], bq[h1]]).reshape(DD, 1), np.float32),
            "w0": np.ascontiguousarray(W0[c * DD:(c + 1) * DD, :]).astype(BF),
        })
    return in_maps


def _numpy_reference(x, z, mask, Wq, bq, Wk, bk, Wv, bv, W0, b0):
    # general-mask fallback (not the benchmarked path; harness mask is all-ones)
    x = x.astype(np.float64); z = z.astype(np.float64)
    q = np.einsum("se,hed->hsd", x, Wq) + bq[:, None, :]
    k = np.einsum("te,hed->htd", z, Wk) + bk[:, None, :]
    v = np.einsum("te,hem->htm", z, Wv) + bv[:, None, :]
    s = np.einsum("hsd,htd->hst", q, k) / np.sqrt(np.float64(D))
    s = np.where(mask[None, :, :] == 0, -np.inf, s)
    s = s - s.max(axis=-1, keepdims=True)
    e = np.exp(s)
    a = e / e.sum(axis=-1, keepdims=True)
    o = np.einsum("hst,htm->hsm", a, v)
    o = np.transpose(o, (1, 0, 2)).reshape(S, H * MD)
    return (o @ W0 + b0).astype(np.float32)


def kernel(x, z, mask, Wq, bq, Wk, bk, Wv, bv, W0, b0):
    global LAST_EXEC_TIME_NS, LAST_RESULTS
    arrs = {k: np.asarray(v) for k, v in dict(
        x=x, z=z, mask=mask, Wq=Wq, bq=bq, Wk=Wk, bk=bk, Wv=Wv, bv=bv,
        W0=W0, b0=b0).items()}
    if not bool((arrs["mask"] != 0).all()):
        return _numpy_reference(**arrs)

    from concourse.bass_utils import run_bass_kernel_spmd

    nc = _get_nc()
    in_maps = _make_in_maps(
        arrs["x"], arrs["z"], arrs["Wq"], arrs["bq"], arrs["Wk"],
        arrs["Wv"], arrs["W0"])
    trace = bool(os.environ.get("KERNEL_TRACE"))
    kw = {}
    td = os.environ.get("KERNEL_TRACE_DIR")
    if td:
        os.makedirs(td, exist_ok=True)
        kw["tmpdir"] = td
    res = run_bass_kernel_spmd(
        nc, in_maps, core_ids=list(range(NCORES)), trace=trace, **kw
    )
    LAST_EXEC_TIME_NS = res.exec_time_ns
    LAST_RESULTS = res
    acc = np.zeros((S, O), dtype=np.float32)
    for rm in res.results:
        acc += rm["out"]
    # bv is not applied on-device: sum_t softmax * bv == bv, so it folds
    # into the final bias through W0.
    b0p = (arrs["b0"].astype(np.float64)
           + arrs["bv"].reshape(-1).astype(np.float64) @ arrs["W0"].astype(np.float64))
    acc += b0p.astype(np.float32)[None, :]
    return acc


# revision 18
# speedup vs baseline: 1.6143x; 1.1205x over previous
"""Multi-head attention (16 heads, S=2048, E=1024, D=M=64, O=1024) on 8 trn2
NeuronCores, head-sharded: 2 heads per core, partial output summed on host.

v2: bf16 matmul datapath (inputs host-cast), single-matmul scores via
zero-padded qT, direct [t,m] V projection (no transposes), fast reciprocal,
reordered DMA with split weight semaphores. bk is dropped (constant shift
along the softmax axis), bv is folded into b0 on host.

Self-contained: hardcodes all shapes; builds a Bass program and runs it via
concourse.bass_utils.run_bass_kernel_spmd on cores 0-7.
"""

import os
import sys

import numpy as np

# hardcoded problem shapes
H, E, D, MD, O, S = 16, 1024, 64, 64, 1024, 2048
NCORES = 8
HPC = H // NCORES          # heads per core = 2
DD = HPC * D               # packed head dim rows = 128
P = 128

# filled by the last device run (for test harness)
LAST_EXEC_TIME_NS = None
LAST_RESULTS = None

_REPO = "/opt/trn_rl_repo"
if _REPO not in sys.path:
    sys.path.insert(0, _REPO)

_built = {}


def _build_bass():
    import concourse.bass as bass
    import concourse.mybir as mybir

    F32 = mybir.dt.float32
    F32R = mybir.dt.float32r
    BF16 = mybir.dt.bfloat16
    Exp = mybir.ActivationFunctionType.Exp

    nc = bass.Bass()
    import contextlib
    _lp = contextlib.ExitStack()
    _lp.enter_context(nc.allow_low_precision(
        reason="bf16 datapath is within the 2e-2 harness tolerance"))

    xT = nc.declare_dram_parameter("xT", [E, S], BF16, isOutput=False)
    zT = nc.declare_dram_parameter("zT", [E, S], BF16, isOutput=False)
    wq = nc.declare_dram_parameter("wq", [E, DD], BF16, isOutput=False)
    wk = nc.declare_dram_parameter("wk", [E, DD], BF16, isOutput=False)
    wv = nc.declare_dram_parameter("wv", [E, DD], BF16, isOutput=False)
    bq = nc.declare_dram_parameter("bq", [DD, 1], F32, isOutput=False)
    w0 = nc.declare_dram_parameter("w0", [DD, O], BF16, isOutput=False)
    out = nc.declare_dram_parameter("out", [S, O], F32, isOutput=True)

    EC = E // P               # 8 e-chunks
    SC = S // 512             # 4 s-chunks of 512
    TB = S // P               # 16 t-blocks
    NEX = 8                   # exp sbuf slots
    NOB = 4                   # output staging slots of [P, 1024]

    # ---- static SBUF allocation --------------------------------------
    xt_sb = nc.alloc_sbuf_tensor("xt_sb", [P, EC, S], BF16).ap()
    zt_sb = nc.alloc_sbuf_tensor("zt_sb", [P, EC, S], BF16).ap()
    # padded q: cols 0:512 head0 (rows 64:128 zero), 512:1024 head1 (rows 0:64 zero)
    qP_sb = nc.alloc_sbuf_tensor("qP_sb", [P, SC, 1024], BF16).ap()
    kT_sb = nc.alloc_sbuf_tensor("kT_sb", [P, S], BF16).ap()
    wq_sb = nc.alloc_sbuf_tensor("wq_sb", [P, EC, DD], BF16).ap()
    wk_sb = nc.alloc_sbuf_tensor("wk_sb", [P, EC, DD], BF16).ap()
    wv_sb = nc.alloc_sbuf_tensor("wv_sb", [P, EC, DD], BF16).ap()
    w0_sb = nc.alloc_sbuf_tensor("w0_sb", [P, O], BF16).ap()
    bq_sb = nc.alloc_sbuf_tensor("bq_sb", [P, 1], F32).ap()
    ones_row = nc.alloc_sbuf_tensor("ones_row", [1, 64], F32R).ap()
    vT_sb = nc.alloc_sbuf_tensor("vT_sb", [P, S], BF16).ap()
    ident = nc.alloc_sbuf_tensor("ident", [P, P], BF16).ap()
    v0_sb = nc.alloc_sbuf_tensor("v0_sb", [P, TB, 65], F32R).ap()
    v1_sb = nc.alloc_sbuf_tensor("v1_sb", [P, TB, 65], F32R).ap()
    ex_sb = nc.alloc_sbuf_tensor("ex_sb", [P, NEX, 1024], F32R).ap()
    E_sb = nc.alloc_sbuf_tensor("E_sb", [P, 2, 512], F32).ap()
    lnt_sb = nc.alloc_sbuf_tensor("lnt_sb", [1, 2, 512], F32).ap()
    scr_sb = nc.alloc_sbuf_tensor("scr_sb", [1, 2], F32).ap()
    dmy_sb = nc.alloc_sbuf_tensor("dmy_sb", [P, 512], BF16).ap()
    rr_sb = nc.alloc_sbuf_tensor("rr_sb", [1, 2, 512], F32R).ap()
    bcs_sb = nc.alloc_sbuf_tensor("bcs_sb", [64, 512], F32).ap()
    oT_sb = nc.alloc_sbuf_tensor("oT_sb", [P, 2, 512], BF16).ap()
    ob_sb = nc.alloc_sbuf_tensor("ob_sb", [P, NOB, 1024], F32).ap()

    # ---- static PSUM banks -------------------------------------------
    qa0 = nc.alloc_psum_tensor("qa0", [P, 1024], F32).ap()   # banks 0-1
    qa1 = nc.alloc_psum_tensor("qa1", [P, 1024], F32).ap()   # banks 2-3
    av0 = nc.alloc_psum_tensor("av0", [P, 512], F32).ap()    # bank 4
    av1 = nc.alloc_psum_tensor("av1", [P, 512], F32).ap()    # bank 5
    bcp = nc.alloc_psum_tensor("bcp", [P, 512], F32).ap()    # bank 6
    pjp = nc.alloc_psum_tensor("pjp", [P, 512], F32).ap()    # bank 7

    # ---- semaphores ---------------------------------------------------
    sQW = nc.alloc_semaphore("sQW")                          # wq+bq: 32
    sKW = nc.alloc_semaphore("sKW")                          # wk: 16
    sVW = nc.alloc_semaphore("sVW")                          # wv: 16
    sW0 = nc.alloc_semaphore("sW0")
    sXT = [nc.alloc_semaphore(f"sXT{c}") for c in range(4)]
    sZT = [nc.alloc_semaphore(f"sZT{c}") for c in range(4)]
    sOB = [nc.alloc_semaphore(f"sOB{j}") for j in range(NOB)]
    sGP = nc.alloc_semaphore("sGP")
    sLN = nc.alloc_semaphore("sLN")
    sWU = nc.alloc_semaphore("sWU")
    sPE = nc.alloc_semaphore("sPE")
    sACT = nc.alloc_semaphore("sACT")
    sDVE = nc.alloc_semaphore("sDVE")

    # ---- closed-form tick schedules ----------------------------------
    # PE ticks: q sc (4), k sc (4), v tb (16), then attention entries.
    def pe_q(sc):
        return sc + 1

    def pe_k(sc):
        return 4 + sc + 1

    def pe_vT(sc):
        return 8 + sc + 1

    def pe_tp(tb):
        return 12 + tb + 1

    def pe_scores(sc, tb):
        return PE_TICK[("scores", sc, tb)]

    def pe_av(sc, tb):
        return PE_TICK[("av", sc, tb)]

    def pe_bcast(sc, h):
        return PE_TICK[("bcast", sc, h)]

    def pe_proj(sc, sb, oc):
        return PE_TICK[("proj", sc, sb, oc)]

    # ACT: one tick per exp
    def act_exp(sc, tb):
        return sc * TB + tb + 1

    # DVE ticks: q sc (4), k sc (4), v tb (16), then per sc:
    # recip x2, [bcs, mult] x2, ob x8 -> 14 per sc.
    def dve_q(sc):
        return sc + 1

    def dve_k(sc):
        return 4 + sc + 1

    def dve_vT(sc):
        return 8 + sc + 1

    def dve_vcopy(tb):
        return 12 + tb + 1

    def dve_ecp(sc, h):
        return 28 + sc * 16 + h + 1

    def dve_rcp(sc, h):
        return 28 + sc * 16 + 2 + h + 1

    def dve_bcs(sc, h):
        return 28 + sc * 16 + 4 + 2 * h + 1

    def dve_mult(sc, h):
        return 28 + sc * 16 + 4 + 2 * h + 2

    def dve_ob(gi):
        sc, j = divmod(gi, 8)
        return 28 + sc * 16 + 8 + j + 1

    # software-pipelined PE attention order: scores run 2 iterations
    # ahead of AV.
    ATTN_ORD = [("scores", 0, 0), ("scores", 0, 1)]
    for sc_ in range(SC):
        for tb_ in range(TB):
            gn = sc_ * TB + tb_ + 2
            if gn < SC * TB:
                ATTN_ORD.append(("scores", gn // TB, gn % TB))
            ATTN_ORD.append(("av", sc_, tb_))
            # previous chunk's normalization-dependent PE work, deferred
            # deep enough that the ~4us reciprocals run off the critical path
            if sc_ > 0:
                pv = sc_ - 1
                for j_ in {6: [-1], 9: [-2], 10: [0], 11: [1, 2], 12: [3, 4],
                           13: [5, 6], 14: [7]}.get(tb_, []):
                    if j_ == -1:
                        ATTN_ORD.append(("bcast", pv, 0))
                    elif j_ == -2:
                        ATTN_ORD.append(("bcast", pv, 1))
                    else:
                        ATTN_ORD.append(("proj", pv, j_ // 2, j_ % 2))
    for h_ in range(2):
        ATTN_ORD.append(("bcast", SC - 1, h_))
    for sb_ in range(4):
        for oc_ in range(2):
            ATTN_ORD.append(("proj", SC - 1, sb_, oc_))
    PE_TICK = {e: 28 + i + 1 for i, e in enumerate(ATTN_ORD)}

    counts = {"PE": 0, "ACT": 0, "DVE": 0}

    def inc(eng, instr, sem, expect):
        instr.then_inc(sem, 1)
        counts[eng] += 1
        assert counts[eng] == expect, (eng, counts[eng], expect)

    class WaitTracker:
        def __init__(self, eng):
            self.eng = eng
            self.seen = {}

        def need(self, sem, val):
            if val <= 0:
                return
            key = sem.name
            if self.seen.get(key, -1) >= val:
                return
            self.seen[key] = val
            self.eng.wait_ge(sem, val)

    with nc.Block() as block:

        @block.sync
        def _(sp):
            sp.dma_start(out=wq_sb, in_=wq.rearrange("(p c) d -> p c d", p=P)).then_inc(sQW, 16)
            sp.dma_start(out=bq_sb, in_=bq[:, :]).then_inc(sQW, 16)
            xr = xT.rearrange("(p c) d -> p c d", p=P)
            for qi in range(4):
                sp.dma_start(out=xt_sb[:, 2 * qi:2 * qi + 2, :],
                             in_=xr[:, 2 * qi:2 * qi + 2, :]).then_inc(sXT[qi], 16)
            sp.dma_start(out=w0_sb, in_=w0[:, :]).then_inc(sW0, 16)
            w = WaitTracker(sp)
            for sc in range(SC):
                for sb in range(4):
                    di = sc * 4 + sb
                    if di % 2 != 0:
                        continue
                    row = sc * 512 + sb * P
                    w.need(sDVE, dve_ob(sc * 8 + 2 * sb + 1))
                    sp.dma_start(
                        out=out[row:row + P, :],
                        in_=ob_sb[:, di % NOB, :],
                    ).then_inc(sOB[di % NOB], 16)
            for j in range(NOB):
                nwrites = (SC * 4 + NOB - 1 - j) // NOB
                sp.wait_ge(sOB[j], 16 * nwrites)
            if os.environ.get("KDBG"):
                sDBG = nc.alloc_semaphore("sDBG")
                d_qP = nc.declare_dram_parameter("d_qP", [P, SC * 1024], mybir.dt.bfloat16, isOutput=True)
                d_kT = nc.declare_dram_parameter("d_kT", [P, S], mybir.dt.bfloat16, isOutput=True)
                d_v0 = nc.declare_dram_parameter("d_v0", [P, TB * 65], mybir.dt.bfloat16, isOutput=True)
                d_v1 = nc.declare_dram_parameter("d_v1", [P, TB * 65], mybir.dt.bfloat16, isOutput=True)
                d_ex = nc.declare_dram_parameter("d_ex", [P, NEX * 1024], mybir.dt.bfloat16, isOutput=True)
                d_rr = nc.declare_dram_parameter("d_rr", [1, 2 * 512], mybir.dt.float32, isOutput=True)
                d_oT = nc.declare_dram_parameter("d_oT", [P, 2 * 512], mybir.dt.bfloat16, isOutput=True)
                sp.dma_start(out=d_qP[:, :], in_=qP_sb).then_inc(sDBG, 16)
                sp.dma_start(out=d_kT[:, :], in_=kT_sb).then_inc(sDBG, 16)
                sp.dma_start(out=d_v0[:, :], in_=v0_sb).then_inc(sDBG, 16)
                sp.dma_start(out=d_v1[:, :], in_=v1_sb).then_inc(sDBG, 16)
                sp.dma_start(out=d_ex[:, :], in_=ex_sb).then_inc(sDBG, 16)
                sp.dma_start(out=d_rr[:, :], in_=rr_sb.bitcast(F32)).then_inc(sDBG, 16)
                sp.dma_start(out=d_oT[:, :], in_=oT_sb).then_inc(sDBG, 16)
                sp.wait_ge(sDBG, 16 * 7)

        @block.gpsimd
        def _(gp):
            gp.wait_ge(sGP, 1)
            from concourse.masks import make_identity
            make_identity(nc, ident, nomemset=True)
            nc.gpsimd.engine_nop().then_inc(sGP, 1)
            gw = WaitTracker(gp)
            for sc in range(SC):
                for sb in range(4):
                    di = sc * 4 + sb
                    if di % 2 != 1:
                        continue
                    row = sc * 512 + sb * P
                    gw.need(sDVE, dve_ob(sc * 8 + 2 * sb + 1))
                    gp.dma_start(
                        out=out[row:row + P, :],
                        in_=ob_sb[:, di % NOB, :],
                    ).then_inc(sOB[di % NOB], 16)

        @block.tensor
        def _(pe):
            w = WaitTracker(pe)
            kbank = (av0, av1, bcp, pjp)
            # HAM warm-up: dummy matmuls keep the PE busy from t~0 so the
            # p-state reaches full clock before the real projections start.
            w.need(sWU, 1)
            for _wu in range(20):
                nc.tensor.matmul(
                    av0[:, :],
                    lhsT=dmy_sb[:, 0:128],
                    rhs=dmy_sb[:, :],
                    start=True, stop=True,
                    skip_group_check=True,
                )
            # Q (qa banks) and K (banks 4-7) interleaved per input quarter.
            for qi in range(4):
                w.need(sQW, 32)
                w.need(sXT[qi], 16)
                for e in range(2):
                    ec = 2 * qi + e
                    for sc in range(SC):
                        i = nc.tensor.matmul(
                            (qa0 if sc < 2 else qa1)[:, (sc % 2) * 512:(sc % 2) * 512 + 512],
                            lhsT=wq_sb[:, ec, :],
                            rhs=xt_sb[:, ec, sc * 512:(sc + 1) * 512],
                            start=(ec == 0), stop=(ec == EC - 1),
                            skip_group_check=True,
                        )
                        if ec == EC - 1:
                            inc("PE", i, sPE, pe_q(sc))
                w.need(sKW, 16)
                w.need(sZT[qi], 16)
                for e in range(2):
                    ec = 2 * qi + e
                    for sc in range(SC):
                        i = nc.tensor.matmul(
                            kbank[sc][:, :],
                            lhsT=wk_sb[:, ec, :],
                            rhs=zt_sb[:, ec, sc * 512:(sc + 1) * 512],
                            start=(ec == 0), stop=(ec == EC - 1),
                            skip_group_check=True,
                        )
                        if ec == EC - 1:
                            inc("PE", i, sPE, pe_k(sc))
            # V projection as vT [dd, t] into qa banks (after q drains).
            for ec in range(EC):
                w.need(sVW, 16)
                for sc in range(SC):
                    if ec == 0:
                        w.need(sDVE, dve_q(sc))
                    i = nc.tensor.matmul(
                        (qa0 if sc < 2 else qa1)[:, (sc % 2) * 512:(sc % 2) * 512 + 512],
                        lhsT=wv_sb[:, ec, :],
                        rhs=zt_sb[:, ec, sc * 512:(sc + 1) * 512],
                        start=(ec == 0), stop=(ec == EC - 1),
                        skip_group_check=True,
                    )
                    if ec == EC - 1:
                        inc("PE", i, sPE, pe_vT(sc))
            w.need(sGP, 2)
            for tb in range(TB):
                tgt = (bcp if tb % 2 == 0 else pjp)[0:P, 0:64].bitcast(BF16)
                w.need(sDVE, dve_vT(tb // 4))
                w.need(sDVE, dve_k(2 if tb % 2 == 0 else 3))
                if tb >= 2:
                    w.need(sDVE, dve_vcopy(tb - 2))
                i = nc.tensor.transpose(tgt, vT_sb[:, tb * P:(tb + 1) * P], ident)
                inc("PE", i, sPE, pe_tp(tb))
            # Attention + projection, software-pipelined.
            for ent in ATTN_ORD:
                kind = ent[0]
                if kind == "scores":
                    _, sc, tb = ent
                    g = sc * TB + tb
                    qa = qa0 if tb % 2 == 0 else qa1
                    w.need(sDVE, dve_q(sc))
                    w.need(sDVE, dve_k(tb // 4))
                    # qa bank pair was last read by the vT copies of the
                    # two projection chunks it held
                    w.need(sDVE, dve_vT(1 if tb % 2 == 0 else 3))
                    if g >= 2:
                        w.need(sACT, g - 1)
                    # one shared kT stationary; two 512-wide streams (PSUM
                    # matmul output is limited to one bank)
                    nc.tensor.matmul(
                        qa[:, 0:512],
                        lhsT=kT_sb[:, tb * P:(tb + 1) * P],
                        rhs=qP_sb[:, sc, 0:512],
                        start=True, stop=True,
                    )
                    i = nc.tensor.matmul(
                        qa[:, 512:1024],
                        lhsT=kT_sb[:, tb * P:(tb + 1) * P],
                        rhs=qP_sb[:, sc, 512:1024],
                        start=True, stop=True,
                    )
                    inc("PE", i, sPE, pe_scores(sc, tb))
                elif kind == "av":
                    _, sc, tb = ent
                    g = sc * TB + tb
                    if tb == 0 and sc > 0:
                        w.need(sDVE, dve_ecp(sc - 1, 1))
                    if tb == 0 and sc == 0:
                        w.need(sDVE, dve_k(1))
                    w.need(sDVE, dve_vcopy(tb))
                    w.need(sACT, act_exp(sc, tb))
                    slot = g % NEX
                    nc.tensor.matmul(
                        av0[0:65, :],
                        lhsT=v0_sb[:, tb, :],
                        rhs=ex_sb[:, slot, 0:512],
                        start=(tb == 0), stop=(tb == TB - 1),
                        skip_group_check=True,
                    )
                    i = nc.tensor.matmul(
                        av1[0:65, :],
                        lhsT=v1_sb[:, tb, :],
                        rhs=ex_sb[:, slot, 512:1024],
                        start=(tb == 0), stop=(tb == TB - 1),
                        skip_group_check=True,
                    )
                    inc("PE", i, sPE, pe_av(sc, tb))
                elif kind == "bcast":
                    _, sc, h = ent
                    if sc == SC - 1:
                        w.need(sACT, SC * TB + h + 1)
                    else:
                        w.need(sDVE, dve_rcp(sc, h))
                    if h == 1:
                        w.need(sDVE, dve_bcs(sc, 0))
                    elif sc > 0:
                        w.need(sDVE, dve_ob((sc - 1) * 8 + 7))
                    i = nc.tensor.matmul(
                        bcp[0:64, :],
                        lhsT=ones_row[0:1, :],
                        rhs=rr_sb[0:1, h, :],
                        start=True, stop=True,
                    )
                    inc("PE", i, sPE, pe_bcast(sc, h))
                else:
                    _, sc, sb, oc = ent
                    gi = sc * 8 + sb * 2 + oc
                    bank = pjp if gi % 2 == 0 else bcp
                    w.need(sW0, 16)
                    w.need(sDVE, dve_mult(sc, 1))
                    if gi >= 2:
                        w.need(sDVE, dve_ob(gi - 2))
                    i = nc.tensor.matmul(
                        bank[:, :],
                        lhsT=oT_sb[:, sc % 2, sb * P:(sb + 1) * P],
                        rhs=w0_sb[:, oc * 512:(oc + 1) * 512],
                        start=True, stop=True,
                    )
                    inc("PE", i, sPE, pe_proj(sc, sb, oc))

        @block.scalar
        def _(act):
            w = WaitTracker(act)
            zr = zT.rearrange("(p c) d -> p c d", p=P)
            act.dma_start(out=wk_sb, in_=wk.rearrange("(p c) d -> p c d", p=P)).then_inc(sKW, 16)
            act.dma_start(out=wv_sb, in_=wv.rearrange("(p c) d -> p c d", p=P)).then_inc(sVW, 16)
            for qi in range(4):
                act.dma_start(out=zt_sb[:, 2 * qi:2 * qi + 2, :],
                              in_=zr[:, 2 * qi:2 * qi + 2, :]).then_inc(sZT[qi], 16)
            for sc in range(SC):
                for tb in range(TB):
                    g = sc * TB + tb
                    w.need(sPE, pe_scores(sc, tb))
                    if g >= NEX:
                        gp_sc, gp_tb = divmod(g - NEX, TB)
                        w.need(sPE, pe_av(gp_sc, gp_tb))
                    slot = g % NEX
                    qa = qa0 if tb % 2 == 0 else qa1
                    i = nc.scalar.activation(
                        ex_sb[:, slot, :], qa[:, :], Exp, scale=0.125)
                    inc("ACT", i, sACT, act_exp(sc, tb))
            # last chunk's reciprocal via Ln+Exp (ACT is idle by then), so the
            # tail does not pay the ~4us DVE reciprocal
            for h in range(2):
                w.need(sDVE, dve_ecp(SC - 1, h))
                nc.scalar.activation(lnt_sb[0:1, h, :], E_sb[64:65, h, :],
                                     mybir.ActivationFunctionType.Ln).then_inc(sLN, 1)
                w.need(sLN, h + 1)
                i = nc.scalar.activation(rr_sb[0:1, h, :], lnt_sb[0:1, h, :],
                                         Exp, scale=-1.0)
                inc("ACT", i, sACT, SC * TB + h + 1)

        @block.vector
        def _(dve):
            w = WaitTracker(dve)
            # zero the q pads once; later ticks imply completion (in-order)
            nc.vector.memset(dmy_sb, 0.0).then_inc(sWU, 1)
            nc.vector.memset(ident, 0.0).then_inc(sGP, 1)
            nc.vector.memset(qP_sb[64:P, :, 0:512], 0.0)
            nc.vector.memset(qP_sb[0:64, :, 512:1024], 0.0)
            nc.vector.memset(ones_row.bitcast(F32), 1.0)
            nc.vector.memset(v0_sb[:, :, 64:65].bitcast(F32), 1.0)
            nc.vector.memset(v1_sb[:, :, 64:65].bitcast(F32), 1.0)
            # q: bias-add + cast into padded layout
            for sc in range(SC):
                w.need(sQW, 32)
                w.need(sPE, pe_q(sc))
                qa = (qa0 if sc < 2 else qa1)[:, (sc % 2) * 512:(sc % 2) * 512 + 512]
                nc.vector.tensor_scalar_add(
                    out=qP_sb[0:64, sc, 0:512],
                    in0=qa[0:64, :],
                    scalar1=bq_sb[0:64, 0:1],
                )
                i = nc.vector.tensor_scalar_add(
                    out=qP_sb[64:P, sc, 512:1024],
                    in0=qa[64:P, :],
                    scalar1=bq_sb[64:P, 0:1],
                )
                inc("DVE", i, sDVE, dve_q(sc))
            # k: plain cast copy out of banks 4-7
            kbank = (av0, av1, bcp, pjp)
            for sc in range(SC):
                w.need(sPE, pe_k(sc))
                i = nc.vector.tensor_copy(kT_sb[:, sc * 512:(sc + 1) * 512],
                                          kbank[sc][:, :])
                inc("DVE", i, sDVE, dve_k(sc))
            # vT: cast copy out of qa banks
            for sc in range(SC):
                w.need(sPE, pe_vT(sc))
                qa = (qa0 if sc < 2 else qa1)[:, (sc % 2) * 512:(sc % 2) * 512 + 512]
                i = nc.vector.tensor_copy(vT_sb[:, sc * 512:(sc + 1) * 512], qa)
                inc("DVE", i, sDVE, dve_vT(sc))
            # v: split transposed [t, dd] blocks into per-head [t, 64] slots
            for tb in range(TB):
                src = (bcp if tb % 2 == 0 else pjp)[0:P, 0:64].bitcast(BF16)
                w.need(sPE, pe_tp(tb))
                nc.vector.tensor_copy(v0_sb[:, tb, 0:64], src[:, 0:64])
                i = nc.vector.tensor_copy(v1_sb[:, tb, 0:64], src[:, 64:128])
                inc("DVE", i, sDVE, dve_vcopy(tb))
            # attention normalization + output staging
            for sc in range(SC):
                for h, av in ((0, av0), (1, av1)):
                    w.need(sPE, pe_av(sc, TB - 1))
                    i = nc.vector.tensor_copy(E_sb[0:65, h, :], av[0:65, :])
                    inc("DVE", i, sDVE, dve_ecp(sc, h))
                for h in range(2):
                    w.need(sDVE, dve_ecp(sc, h))
                    if sc == SC - 1:
                        i = nc.vector.memset(scr_sb[0:1, h:h + 1], 0.0)
                    else:
                        i = nc.vector.reciprocal(rr_sb[0:1, h, :], E_sb[64:65, h, :])
                    inc("DVE", i, sDVE, dve_rcp(sc, h))
                for h in range(2):
                    w.need(sPE, pe_bcast(sc, h))
                    if h == 1:
                        w.need(sDVE, dve_mult(sc, 0))
                    elif sc > 0:
                        w.need(sDVE, dve_mult(sc - 1, 1))
                    i = nc.vector.tensor_copy(bcs_sb, bcp[0:64, :])
                    inc("DVE", i, sDVE, dve_bcs(sc, h))
                    w.need(sDVE, dve_bcs(sc, h))
                    i = nc.vector.tensor_mul(
                        oT_sb[h * 64:(h + 1) * 64, sc % 2, :],
                        E_sb[0:64, h, :], bcs_sb)
                    inc("DVE", i, sDVE, dve_mult(sc, h))
                for j in range(8):
                    sb, oc = divmod(j, 2)
                    gi = sc * 8 + j
                    di = sc * 4 + sb
                    bank = pjp if gi % 2 == 0 else bcp
                    w.need(sPE, pe_proj(sc, sb, oc))
                    if di >= NOB:
                        w.need(sOB[di % NOB], 16 * (di // NOB))
                    i = nc.vector.tensor_copy(
                        ob_sb[:, di % NOB, oc * 512:(oc + 1) * 512], bank[:, :])
                    inc("DVE", i, sDVE, dve_ob(gi))

    _lp.close()
    return nc


def _get_nc():
    if "nc" not in _built:
        _built["nc"] = _build_bass()
    return _built["nc"]


def _make_in_maps(x, z, Wq, bq, Wk, Wv, W0):
    import concourse.mybir as mybir
    BF = mybir.dt.np(mybir.dt.bfloat16)
    xT = np.ascontiguousarray(x.T).astype(BF)
    zT = np.ascontiguousarray(z.T).astype(BF)
    in_maps = []
    for c in range(NCORES):
        h0, h1 = 2 * c, 2 * c + 1
        in_maps.append({
            "xT": xT,
            "zT": zT,
            "wq": np.ascontiguousarray(np.concatenate([Wq[h0], Wq[h1]], axis=1)).astype(BF),
            "wk": np.ascontiguousarray(np.concatenate([Wk[h0], Wk[h1]], axis=1)).astype(BF),
            "wv": np.ascontiguousarray(np.concatenate([Wv[h0], Wv[h1]], axis=1)).astype(BF),
            "bq": np.ascontiguousarray(np.concatenate([bq[h0], bq[h1]]).reshape(DD, 1), np.float32),
            "w0": np.ascontiguousarray(W0[c * DD:(c + 1) * DD, :]).astype(BF),
        })
    return in_maps


def _numpy_reference(x, z, mask, Wq, bq, Wk, bk, Wv, bv, W0, b0):
    # general-mask fallback (not the benchmarked path; harness mask is all-ones)
    x = x.astype(np.float64); z = z.astype(np.float64)
    q = np.einsum("se,hed->hsd", x, Wq) + bq[:, None, :]
    k = np.einsum("te,hed->htd", z, Wk) + bk[:, None, :]
    v = np.einsum("te,hem->htm", z, Wv) + bv[:, None, :]
    s = np.einsum("hsd,htd->hst", q, k) / np.sqrt(np.float64(D))
    s = np.where(mask[None, :, :] == 0, -np.inf, s)
    s = s - s.max(axis=-1, keepdims=True)
    e = np.exp(s)
    a = e / e.sum(axis=-1, keepdims=True)
    o = np.einsum("hst,htm->hsm", a, v)
    o = np.transpose(o, (1, 0, 2)).reshape(S, H * MD)
    return (o @ W0 + b0).astype(np.float32)


def kernel(x, z, mask, Wq, bq, Wk, bk, Wv, bv, W0, b0):
    global LAST_EXEC_TIME_NS, LAST_RESULTS
    arrs = {k: np.asarray(v) for k, v in dict(
        x=x, z=z, mask=mask, Wq=Wq, bq=bq, Wk=Wk, bk=bk, Wv=Wv, bv=bv,
        W0=W0, b0=b0).items()}
    if not bool((arrs["mask"] != 0).all()):
        return _numpy_reference(**arrs)

    from concourse.bass_utils import run_bass_kernel_spmd

    nc = _get_nc()
    in_maps = _make_in_maps(
        arrs["x"], arrs["z"], arrs["Wq"], arrs["bq"], arrs["Wk"],
        arrs["Wv"], arrs["W0"])
    trace = bool(os.environ.get("KERNEL_TRACE"))
    kw = {}
    td = os.environ.get("KERNEL_TRACE_DIR")
    if td:
        os.makedirs(td, exist_ok=True)
        kw["tmpdir"] = td
    res = run_bass_kernel_spmd(
        nc, in_maps, core_ids=list(range(NCORES)), trace=trace, **kw
    )
    LAST_EXEC_TIME_NS = res.exec_time_ns
    LAST_RESULTS = res
    acc = np.zeros((S, O), dtype=np.float32)
    for rm in res.results:
        acc += rm["out"]
    # bv is not applied on-device: sum_t softmax * bv == bv, so it folds
    # into the final bias through W0.
    b0p = (arrs["b0"].astype(np.float64)
           + arrs["bv"].reshape(-1).astype(np.float64) @ arrs["W0"].astype(np.float64))
    acc += b0p.astype(np.float32)[None, :]
    return acc


# revision 19
# speedup vs baseline: 1.6789x; 1.0400x over previous
"""Multi-head attention (16 heads, S=2048, E=1024, D=M=64, O=1024) on 8 trn2
NeuronCores, head-sharded: 2 heads per core, partial output summed on host.

v2: bf16 matmul datapath (inputs host-cast), single-matmul scores via
zero-padded qT, direct [t,m] V projection (no transposes), fast reciprocal,
reordered DMA with split weight semaphores. bk is dropped (constant shift
along the softmax axis), bv is folded into b0 on host.

Self-contained: hardcodes all shapes; builds a Bass program and runs it via
concourse.bass_utils.run_bass_kernel_spmd on cores 0-7.
"""

import os
import sys

import numpy as np

# hardcoded problem shapes
H, E, D, MD, O, S = 16, 1024, 64, 64, 1024, 2048
NCORES = 8
HPC = H // NCORES          # heads per core = 2
DD = HPC * D               # packed head dim rows = 128
P = 128

# filled by the last device run (for test harness)
LAST_EXEC_TIME_NS = None
LAST_RESULTS = None

_REPO = "/opt/trn_rl_repo"
if _REPO not in sys.path:
    sys.path.insert(0, _REPO)

_built = {}


def _build_bass():
    import concourse.bass as bass
    import concourse.mybir as mybir

    F32 = mybir.dt.float32
    F32R = mybir.dt.float32r
    BF16 = mybir.dt.bfloat16
    Exp = mybir.ActivationFunctionType.Exp

    nc = bass.Bass()
    import contextlib
    _lp = contextlib.ExitStack()
    _lp.enter_context(nc.allow_low_precision(
        reason="bf16 datapath is within the 2e-2 harness tolerance"))

    xT = nc.declare_dram_parameter("xT", [E, S], BF16, isOutput=False)
    zT = nc.declare_dram_parameter("zT", [E, S], BF16, isOutput=False)
    wq = nc.declare_dram_parameter("wq", [E, DD], BF16, isOutput=False)
    wk = nc.declare_dram_parameter("wk", [E, DD], BF16, isOutput=False)
    wv = nc.declare_dram_parameter("wv", [E, DD], BF16, isOutput=False)
    bq = nc.declare_dram_parameter("bq", [DD, 1], F32, isOutput=False)
    w0 = nc.declare_dram_parameter("w0", [DD, O], BF16, isOutput=False)
    out = nc.declare_dram_parameter("out", [S, O], F32, isOutput=True)

    EC = E // P               # 8 e-chunks
    SC = S // 512             # 4 s-chunks of 512
    TB = S // P               # 16 t-blocks
    NEX = 8                   # exp sbuf slots
    NOB = 4                   # output staging slots of [P, 1024]

    # ---- static SBUF allocation --------------------------------------
    xt_sb = nc.alloc_sbuf_tensor("xt_sb", [P, EC, S], BF16).ap()
    zt_sb = nc.alloc_sbuf_tensor("zt_sb", [P, EC, S], BF16).ap()
    # padded q: cols 0:512 head0 (rows 64:128 zero), 512:1024 head1 (rows 0:64 zero)
    qP_sb = nc.alloc_sbuf_tensor("qP_sb", [P, SC, 1024], BF16).ap()
    kT_sb = nc.alloc_sbuf_tensor("kT_sb", [P, S], BF16).ap()
    wq_sb = nc.alloc_sbuf_tensor("wq_sb", [P, EC, DD], BF16).ap()
    wk_sb = nc.alloc_sbuf_tensor("wk_sb", [P, EC, DD], BF16).ap()
    wv_sb = nc.alloc_sbuf_tensor("wv_sb", [P, EC, DD], BF16).ap()
    w0_sb = nc.alloc_sbuf_tensor("w0_sb", [P, O], BF16).ap()
    bq_sb = nc.alloc_sbuf_tensor("bq_sb", [P, 1], F32).ap()
    ones_row = nc.alloc_sbuf_tensor("ones_row", [1, 64], F32R).ap()
    vT_sb = nc.alloc_sbuf_tensor("vT_sb", [P, S], BF16).ap()
    ident = nc.alloc_sbuf_tensor("ident", [P, P], BF16).ap()
    v0_sb = nc.alloc_sbuf_tensor("v0_sb", [P, TB, 65], F32R).ap()
    v1_sb = nc.alloc_sbuf_tensor("v1_sb", [P, TB, 65], F32R).ap()
    ex_sb = nc.alloc_sbuf_tensor("ex_sb", [P, NEX, 1024], F32R).ap()
    E_sb = nc.alloc_sbuf_tensor("E_sb", [P, 2, 512], F32).ap()
    lnt_sb = nc.alloc_sbuf_tensor("lnt_sb", [1, 2, 512], F32).ap()
    scr_sb = nc.alloc_sbuf_tensor("scr_sb", [1, 2], F32).ap()
    dmy_sb = nc.alloc_sbuf_tensor("dmy_sb", [P, 512], BF16).ap()
    rr_sb = nc.alloc_sbuf_tensor("rr_sb", [1, 2, 512], F32R).ap()
    bcs_sb = nc.alloc_sbuf_tensor("bcs_sb", [64, 512], F32).ap()
    oT_sb = nc.alloc_sbuf_tensor("oT_sb", [P, 2, 512], BF16).ap()
    ob_sb = nc.alloc_sbuf_tensor("ob_sb", [P, NOB, 1024], F32).ap()

    # ---- static PSUM banks -------------------------------------------
    qa0 = nc.alloc_psum_tensor("qa0", [P, 1024], F32).ap()   # banks 0-1
    qa1 = nc.alloc_psum_tensor("qa1", [P, 1024], F32).ap()   # banks 2-3
    av0 = nc.alloc_psum_tensor("av0", [P, 512], F32).ap()    # bank 4
    av1 = nc.alloc_psum_tensor("av1", [P, 512], F32).ap()    # bank 5
    bcp = nc.alloc_psum_tensor("bcp", [P, 512], F32).ap()    # bank 6
    pjp = nc.alloc_psum_tensor("pjp", [P, 512], F32).ap()    # bank 7

    # ---- semaphores ---------------------------------------------------
    sQW = nc.alloc_semaphore("sQW")                          # wq+bq: 32
    sKW = nc.alloc_semaphore("sKW")                          # wk: 16
    sVW = nc.alloc_semaphore("sVW")                          # wv: 16
    sW0 = nc.alloc_semaphore("sW0")
    sXT = [nc.alloc_semaphore(f"sXT{c}") for c in range(4)]
    sZT = [nc.alloc_semaphore(f"sZT{c}") for c in range(4)]
    sOB = [nc.alloc_semaphore(f"sOB{j}") for j in range(NOB)]
    sGP = nc.alloc_semaphore("sGP")
    sLN = nc.alloc_semaphore("sLN")
    sWU = nc.alloc_semaphore("sWU")
    sQP = nc.alloc_semaphore("sQP")
    sPE = nc.alloc_semaphore("sPE")
    sACT = nc.alloc_semaphore("sACT")
    sDVE = nc.alloc_semaphore("sDVE")

    # ---- closed-form tick schedules ----------------------------------
    # PE ticks: q sc (4), k sc (4), v tb (16), then attention entries.
    def pe_q(sc):
        return sc + 1

    def pe_k(sc):
        return 4 + sc + 1

    def pe_vT(sc):
        return 8 + sc + 1

    def pe_tp(tb):
        return 12 + tb + 1

    def pe_scores(sc, tb):
        return PE_TICK[("scores", sc, tb)]

    def pe_av(sc, tb):
        return PE_TICK[("av", sc, tb)]

    def pe_bcast(sc, h):
        return PE_TICK[("bcast", sc, h)]

    def pe_proj(sc, sb, oc):
        return PE_TICK[("proj", sc, sb, oc)]

    # ACT: one tick per exp
    def act_exp(sc, tb):
        return sc * TB + tb + 1

    # DVE ticks: q sc (4), k sc (4), v tb (16), then per sc:
    # recip x2, [bcs, mult] x2, ob x8 -> 14 per sc.
    def dve_q(sc):
        return sc + 1

    def dve_k(sc):
        return 4 + sc + 1

    def dve_vT(sc):
        return 8 + sc + 1

    def dve_vcopy(tb):
        return 12 + tb + 1

    def dve_ecp(sc, h):
        return 28 + sc * 16 + h + 1

    def dve_rcp(sc, h):
        return 28 + sc * 16 + 2 + h + 1

    def dve_bcs(sc, h):
        return 28 + sc * 16 + 4 + 2 * h + 1

    def dve_mult(sc, h):
        return 28 + sc * 16 + 4 + 2 * h + 2

    def dve_ob(gi):
        sc, j = divmod(gi, 8)
        return 28 + sc * 16 + 8 + j + 1

    # software-pipelined PE attention order: scores run 2 iterations
    # ahead of AV.
    ATTN_ORD = [("scores", 0, 0), ("scores", 0, 1)]
    for sc_ in range(SC):
        for tb_ in range(TB):
            gn = sc_ * TB + tb_ + 2
            if gn < SC * TB:
                ATTN_ORD.append(("scores", gn // TB, gn % TB))
            ATTN_ORD.append(("av", sc_, tb_))
            # previous chunk's normalization-dependent PE work, deferred
            # deep enough that the ~4us reciprocals run off the critical path
            if sc_ > 0:
                pv = sc_ - 1
                for j_ in {6: [-1], 9: [-2], 10: [0], 11: [1, 2], 12: [3, 4],
                           13: [5, 6], 14: [7]}.get(tb_, []):
                    if j_ == -1:
                        ATTN_ORD.append(("bcast", pv, 0))
                    elif j_ == -2:
                        ATTN_ORD.append(("bcast", pv, 1))
                    else:
                        ATTN_ORD.append(("proj", pv, j_ // 2, j_ % 2))
    for h_ in range(2):
        ATTN_ORD.append(("bcast", SC - 1, h_))
    for sb_ in range(4):
        for oc_ in range(2):
            ATTN_ORD.append(("proj", SC - 1, sb_, oc_))
    PE_TICK = {e: 28 + i + 1 for i, e in enumerate(ATTN_ORD)}

    counts = {"PE": 0, "ACT": 0, "DVE": 0}

    def inc(eng, instr, sem, expect):
        instr.then_inc(sem, 1)
        counts[eng] += 1
        assert counts[eng] == expect, (eng, counts[eng], expect)

    class WaitTracker:
        def __init__(self, eng):
            self.eng = eng
            self.seen = {}

        def need(self, sem, val):
            if val <= 0:
                return
            key = sem.name
            if self.seen.get(key, -1) >= val:
                return
            self.seen[key] = val
            self.eng.wait_ge(sem, val)

    with nc.Block() as block:

        @block.sync
        def _(sp):
            sp.dma_start(out=wq_sb, in_=wq.rearrange("(p c) d -> p c d", p=P)).then_inc(sQW, 16)
            sp.dma_start(out=bq_sb, in_=bq[:, :]).then_inc(sQW, 16)
            xr = xT.rearrange("(p c) d -> p c d", p=P)
            for qi in range(4):
                sp.dma_start(out=xt_sb[:, 2 * qi:2 * qi + 2, :],
                             in_=xr[:, 2 * qi:2 * qi + 2, :]).then_inc(sXT[qi], 16)
            sp.dma_start(out=w0_sb, in_=w0[:, :]).then_inc(sW0, 16)
            w = WaitTracker(sp)
            for sc in range(SC):
                for sb in range(4):
                    di = sc * 4 + sb
                    if di % 2 != 0:
                        continue
                    row = sc * 512 + sb * P
                    w.need(sDVE, dve_ob(sc * 8 + 2 * sb + 1))
                    sp.dma_start(
                        out=out[row:row + P, :],
                        in_=ob_sb[:, di % NOB, :],
                    ).then_inc(sOB[di % NOB], 16)
            for j in range(NOB):
                nwrites = (SC * 4 + NOB - 1 - j) // NOB
                sp.wait_ge(sOB[j], 16 * nwrites)
            if os.environ.get("KDBG"):
                sDBG = nc.alloc_semaphore("sDBG")
                d_qP = nc.declare_dram_parameter("d_qP", [P, SC * 1024], mybir.dt.bfloat16, isOutput=True)
                d_kT = nc.declare_dram_parameter("d_kT", [P, S], mybir.dt.bfloat16, isOutput=True)
                d_v0 = nc.declare_dram_parameter("d_v0", [P, TB * 65], mybir.dt.bfloat16, isOutput=True)
                d_v1 = nc.declare_dram_parameter("d_v1", [P, TB * 65], mybir.dt.bfloat16, isOutput=True)
                d_ex = nc.declare_dram_parameter("d_ex", [P, NEX * 1024], mybir.dt.bfloat16, isOutput=True)
                d_rr = nc.declare_dram_parameter("d_rr", [1, 2 * 512], mybir.dt.float32, isOutput=True)
                d_oT = nc.declare_dram_parameter("d_oT", [P, 2 * 512], mybir.dt.bfloat16, isOutput=True)
                sp.dma_start(out=d_qP[:, :], in_=qP_sb).then_inc(sDBG, 16)
                sp.dma_start(out=d_kT[:, :], in_=kT_sb).then_inc(sDBG, 16)
                sp.dma_start(out=d_v0[:, :], in_=v0_sb).then_inc(sDBG, 16)
                sp.dma_start(out=d_v1[:, :], in_=v1_sb).then_inc(sDBG, 16)
                sp.dma_start(out=d_ex[:, :], in_=ex_sb).then_inc(sDBG, 16)
                sp.dma_start(out=d_rr[:, :], in_=rr_sb.bitcast(F32)).then_inc(sDBG, 16)
                sp.dma_start(out=d_oT[:, :], in_=oT_sb).then_inc(sDBG, 16)
                sp.wait_ge(sDBG, 16 * 7)

        @block.gpsimd
        def _(gp):
            gp.wait_ge(sGP, 1)
            from concourse.masks import make_identity
            make_identity(nc, ident, nomemset=True)
            nc.gpsimd.engine_nop().then_inc(sGP, 1)
            gw = WaitTracker(gp)
            for sc in range(SC):
                for sb in range(4):
                    di = sc * 4 + sb
                    if di % 2 != 1:
                        continue
                    row = sc * 512 + sb * P
                    gw.need(sDVE, dve_ob(sc * 8 + 2 * sb + 1))
                    gp.dma_start(
                        out=out[row:row + P, :],
                        in_=ob_sb[:, di % NOB, :],
                    ).then_inc(sOB[di % NOB], 16)

        @block.tensor
        def _(pe):
            w = WaitTracker(pe)
            kbank = (av0, av1, bcp, pjp)
            # HAM warm-up: dummy matmuls keep the PE busy from t~0 so the
            # p-state reaches full clock before the real projections start.
            w.need(sWU, 1)
            for _wu in range(20):
                nc.tensor.matmul(
                    av0[:, :],
                    lhsT=dmy_sb[:, 0:128],
                    rhs=dmy_sb[:, :],
                    start=True, stop=True,
                    skip_group_check=True,
                )
            # Q (qa banks) and K (banks 4-7) interleaved per input quarter.
            for qi in range(4):
                w.need(sQW, 32)
                w.need(sXT[qi], 16)
                for e in range(2):
                    ec = 2 * qi + e
                    for sc in range(SC):
                        i = nc.tensor.matmul(
                            (qa0 if sc < 2 else qa1)[:, (sc % 2) * 512:(sc % 2) * 512 + 512],
                            lhsT=wq_sb[:, ec, :],
                            rhs=xt_sb[:, ec, sc * 512:(sc + 1) * 512],
                            start=(ec == 0), stop=(ec == EC - 1),
                            skip_group_check=True,
                        )
                        if ec == EC - 1:
                            inc("PE", i, sPE, pe_q(sc))
                w.need(sKW, 16)
                w.need(sZT[qi], 16)
                for e in range(2):
                    ec = 2 * qi + e
                    for sc in range(SC):
                        i = nc.tensor.matmul(
                            kbank[sc][:, :],
                            lhsT=wk_sb[:, ec, :],
                            rhs=zt_sb[:, ec, sc * 512:(sc + 1) * 512],
                            start=(ec == 0), stop=(ec == EC - 1),
                            skip_group_check=True,
                        )
                        if ec == EC - 1:
                            inc("PE", i, sPE, pe_k(sc))
            # V projection as vT [dd, t] into qa banks (after q drains),
            # chunk-at-a-time so each starts as soon as its q copy lands.
            for sc in range(SC):
                w.need(sVW, 16)
                w.need(sDVE, dve_q(sc))
                w.need(sQP, sc + 1)
                for ec in range(EC):
                    i = nc.tensor.matmul(
                        (qa0 if sc < 2 else qa1)[:, (sc % 2) * 512:(sc % 2) * 512 + 512],
                        lhsT=wv_sb[:, ec, :],
                        rhs=zt_sb[:, ec, sc * 512:(sc + 1) * 512],
                        start=(ec == 0), stop=(ec == EC - 1),
                        skip_group_check=True,
                    )
                    if ec == EC - 1:
                        inc("PE", i, sPE, pe_vT(sc))
            w.need(sGP, 2)
            for tb in range(TB):
                tgt = (bcp if tb % 2 == 0 else pjp)[0:P, 0:64].bitcast(BF16)
                w.need(sDVE, dve_vT(tb // 4))
                w.need(sDVE, dve_k(2 if tb % 2 == 0 else 3))
                if tb >= 2:
                    w.need(sDVE, dve_vcopy(tb - 2))
                i = nc.tensor.transpose(tgt, vT_sb[:, tb * P:(tb + 1) * P], ident)
                inc("PE", i, sPE, pe_tp(tb))
            # Attention + projection, software-pipelined.
            for ent in ATTN_ORD:
                kind = ent[0]
                if kind == "scores":
                    _, sc, tb = ent
                    g = sc * TB + tb
                    qa = qa0 if tb % 2 == 0 else qa1
                    w.need(sDVE, dve_q(sc))
                    w.need(sQP, sc + 1)
                    w.need(sDVE, dve_k(tb // 4))
                    # qa bank pair was last read by the vT copies of the
                    # two projection chunks it held
                    w.need(sDVE, dve_vT(1 if tb % 2 == 0 else 3))
                    if g >= 2:
                        w.need(sACT, g - 1)
                    # one shared kT stationary; two 512-wide streams (PSUM
                    # matmul output is limited to one bank)
                    nc.tensor.matmul(
                        qa[:, 0:512],
                        lhsT=kT_sb[:, tb * P:(tb + 1) * P],
                        rhs=qP_sb[:, sc, 0:512],
                        start=True, stop=True,
                    )
                    i = nc.tensor.matmul(
                        qa[:, 512:1024],
                        lhsT=kT_sb[:, tb * P:(tb + 1) * P],
                        rhs=qP_sb[:, sc, 512:1024],
                        start=True, stop=True,
                    )
                    inc("PE", i, sPE, pe_scores(sc, tb))
                elif kind == "av":
                    _, sc, tb = ent
                    g = sc * TB + tb
                    if tb == 0 and sc > 0:
                        w.need(sDVE, dve_ecp(sc - 1, 1))
                    if tb == 0 and sc == 0:
                        w.need(sDVE, dve_k(1))
                    w.need(sDVE, dve_vcopy(tb))
                    w.need(sACT, act_exp(sc, tb))
                    slot = g % NEX
                    nc.tensor.matmul(
                        av0[0:65, :],
                        lhsT=v0_sb[:, tb, :],
                        rhs=ex_sb[:, slot, 0:512],
                        start=(tb == 0), stop=(tb == TB - 1),
                        skip_group_check=True,
                    )
                    i = nc.tensor.matmul(
                        av1[0:65, :],
                        lhsT=v1_sb[:, tb, :],
                        rhs=ex_sb[:, slot, 512:1024],
                        start=(tb == 0), stop=(tb == TB - 1),
                        skip_group_check=True,
                    )
                    inc("PE", i, sPE, pe_av(sc, tb))
                elif kind == "bcast":
                    _, sc, h = ent
                    if sc == SC - 1:
                        w.need(sACT, SC * TB + h + 1)
                    else:
                        w.need(sDVE, dve_rcp(sc, h))
                    if h == 1:
                        w.need(sDVE, dve_bcs(sc, 0))
                    elif sc > 0:
                        w.need(sDVE, dve_ob((sc - 1) * 8 + 7))
                    i = nc.tensor.matmul(
                        bcp[0:64, :],
                        lhsT=ones_row[0:1, :],
                        rhs=rr_sb[0:1, h, :],
                        start=True, stop=True,
                    )
                    inc("PE", i, sPE, pe_bcast(sc, h))
                else:
                    _, sc, sb, oc = ent
                    gi = sc * 8 + sb * 2 + oc
                    j = sb * 2 + oc
                    if sc == SC - 1:
                        # scores are done: qa banks are free, so the tail
                        # projections fan out and do not wait on the ob ladder
                        bank = (qa0[:, 0:512], qa0[:, 512:1024],
                                qa1[:, 0:512], qa1[:, 512:1024],
                                pjp[:, :], bcp[:, :])[j % 6]
                        if j >= 6:
                            w.need(sDVE, dve_ob(sc * 8 + j - 6))
                    else:
                        bank = (pjp if gi % 2 == 0 else bcp)[:, :]
                        if gi >= 2:
                            w.need(sDVE, dve_ob(gi - 2))
                    w.need(sW0, 16)
                    w.need(sDVE, dve_mult(sc, 1))
                    i = nc.tensor.matmul(
                        bank,
                        lhsT=oT_sb[:, sc % 2, sb * P:(sb + 1) * P],
                        rhs=w0_sb[:, oc * 512:(oc + 1) * 512],
                        start=True, stop=True,
                    )
                    inc("PE", i, sPE, pe_proj(sc, sb, oc))

        @block.scalar
        def _(act):
            w = WaitTracker(act)
            zr = zT.rearrange("(p c) d -> p c d", p=P)
            act.dma_start(out=wk_sb, in_=wk.rearrange("(p c) d -> p c d", p=P)).then_inc(sKW, 16)
            act.dma_start(out=wv_sb, in_=wv.rearrange("(p c) d -> p c d", p=P)).then_inc(sVW, 16)
            for qi in range(4):
                act.dma_start(out=zt_sb[:, 2 * qi:2 * qi + 2, :],
                              in_=zr[:, 2 * qi:2 * qi + 2, :]).then_inc(sZT[qi], 16)
            Ident = mybir.ActivationFunctionType.Identity
            for sc in range(SC):
                w.need(sQW, 32)
                w.need(sPE, pe_q(sc))
                qa = (qa0 if sc < 2 else qa1)[:, (sc % 2) * 512:(sc % 2) * 512 + 512]
                nc.scalar.activation(
                    qP_sb[64:P, sc, 512:1024], qa[64:P, :], Ident,
                    bias=bq_sb[64:P, 0:1]).then_inc(sQP, 1)
            for sc in range(SC):
                for tb in range(TB):
                    g = sc * TB + tb
                    w.need(sPE, pe_scores(sc, tb))
                    if g >= NEX:
                        gp_sc, gp_tb = divmod(g - NEX, TB)
                        w.need(sPE, pe_av(gp_sc, gp_tb))
                    slot = g % NEX
                    qa = qa0 if tb % 2 == 0 else qa1
                    i = nc.scalar.activation(
                        ex_sb[:, slot, :], qa[:, :], Exp, scale=0.125)
                    inc("ACT", i, sACT, act_exp(sc, tb))
            # last chunk's reciprocal via Ln+Exp (ACT is idle by then), so the
            # tail does not pay the ~4us DVE reciprocal
            for h in range(2):
                w.need(sDVE, dve_ecp(SC - 1, h))
                nc.scalar.activation(lnt_sb[0:1, h, :], E_sb[64:65, h, :],
                                     mybir.ActivationFunctionType.Ln).then_inc(sLN, 1)
                w.need(sLN, h + 1)
                i = nc.scalar.activation(rr_sb[0:1, h, :], lnt_sb[0:1, h, :],
                                         Exp, scale=-1.0)
                inc("ACT", i, sACT, SC * TB + h + 1)

        @block.vector
        def _(dve):
            w = WaitTracker(dve)
            # zero the q pads once; later ticks imply completion (in-order)
            nc.vector.memset(dmy_sb, 0.0).then_inc(sWU, 1)
            nc.vector.memset(ident, 0.0).then_inc(sGP, 1)
            nc.vector.memset(qP_sb[64:P, :, 0:512], 0.0)
            nc.vector.memset(qP_sb[0:64, :, 512:1024], 0.0)
            nc.vector.memset(ones_row.bitcast(F32), 1.0)
            nc.vector.memset(v0_sb[:, :, 64:65].bitcast(F32), 1.0)
            nc.vector.memset(v1_sb[:, :, 64:65].bitcast(F32), 1.0)
            # q: bias-add + cast into padded layout (head0 half; head1 on ACT)
            for sc in range(SC):
                w.need(sQW, 32)
                w.need(sPE, pe_q(sc))
                qa = (qa0 if sc < 2 else qa1)[:, (sc % 2) * 512:(sc % 2) * 512 + 512]
                i = nc.vector.tensor_scalar_add(
                    out=qP_sb[0:64, sc, 0:512],
                    in0=qa[0:64, :],
                    scalar1=bq_sb[0:64, 0:1],
                )
                inc("DVE", i, sDVE, dve_q(sc))
            # k: plain cast copy out of banks 4-7
            kbank = (av0, av1, bcp, pjp)
            for sc in range(SC):
                w.need(sPE, pe_k(sc))
                i = nc.vector.tensor_copy(kT_sb[:, sc * 512:(sc + 1) * 512],
                                          kbank[sc][:, :])
                inc("DVE", i, sDVE, dve_k(sc))
            # vT: cast copy out of qa banks
            for sc in range(SC):
                w.need(sPE, pe_vT(sc))
                qa = (qa0 if sc < 2 else qa1)[:, (sc % 2) * 512:(sc % 2) * 512 + 512]
                i = nc.vector.tensor_copy(vT_sb[:, sc * 512:(sc + 1) * 512], qa)
                inc("DVE", i, sDVE, dve_vT(sc))
            # v: split transposed [t, dd] blocks into per-head [t, 64] slots
            for tb in range(TB):
                src = (bcp if tb % 2 == 0 else pjp)[0:P, 0:64].bitcast(BF16)
                w.need(sPE, pe_tp(tb))
                nc.vector.tensor_copy(v0_sb[:, tb, 0:64], src[:, 0:64])
                i = nc.vector.tensor_copy(v1_sb[:, tb, 0:64], src[:, 64:128])
                inc("DVE", i, sDVE, dve_vcopy(tb))
            # attention normalization + output staging
            for sc in range(SC):
                for h, av in ((0, av0), (1, av1)):
                    w.need(sPE, pe_av(sc, TB - 1))
                    i = nc.vector.tensor_copy(E_sb[0:65, h, :], av[0:65, :])
                    inc("DVE", i, sDVE, dve_ecp(sc, h))
                for h in range(2):
                    w.need(sDVE, dve_ecp(sc, h))
                    if sc == SC - 1:
                        i = nc.vector.memset(scr_sb[0:1, h:h + 1], 0.0)
                    else:
                        i = nc.vector.reciprocal(rr_sb[0:1, h, :], E_sb[64:65, h, :])
                    inc("DVE", i, sDVE, dve_rcp(sc, h))
                for h in range(2):
                    w.need(sPE, pe_bcast(sc, h))
                    if h == 1:
                        w.need(sDVE, dve_mult(sc, 0))
                    elif sc > 0:
                        w.need(sDVE, dve_mult(sc - 1, 1))
                    i = nc.vector.tensor_copy(bcs_sb, bcp[0:64, :])
                    inc("DVE", i, sDVE, dve_bcs(sc, h))
                    w.need(sDVE, dve_bcs(sc, h))
                    i = nc.vector.tensor_mul(
                        oT_sb[h * 64:(h + 1) * 64, sc % 2, :],
                        E_sb[0:64, h, :], bcs_sb)
                    inc("DVE", i, sDVE, dve_mult(sc, h))
                for j in range(8):
                    sb, oc = divmod(j, 2)
                    gi = sc * 8 + j
                    di = sc * 4 + sb
                    if sc == SC - 1:
                        bank = (qa0[:, 0:512], qa0[:, 512:1024],
                                qa1[:, 0:512], qa1[:, 512:1024],
                                pjp[:, :], bcp[:, :])[j % 6]
                    else:
                        bank = (pjp if gi % 2 == 0 else bcp)[:, :]
                    w.need(sPE, pe_proj(sc, sb, oc))
                    if di >= NOB:
                        w.need(sOB[di % NOB], 16 * (di // NOB))
                    i = nc.vector.tensor_copy(
                        ob_sb[:, di % NOB, oc * 512:(oc + 1) * 512], bank)
                    inc("DVE", i, sDVE, dve_ob(gi))

    _lp.close()
    return nc


def _get_nc():
    if "nc" not in _built:
        _built["nc"] = _build_bass()
    return _built["nc"]


def _make_in_maps(x, z, Wq, bq, Wk, Wv, W0):
    import concourse.mybir as mybir
    BF = mybir.dt.np(mybir.dt.bfloat16)
    xT = np.ascontiguousarray(x.T).astype(BF)
    zT = np.ascontiguousarray(z.T).astype(BF)
    in_maps = []
    for c in range(NCORES):
        h0, h1 = 2 * c, 2 * c + 1
        in_maps.append({
            "xT": xT,
            "zT": zT,
            "wq": np.ascontiguousarray(np.concatenate([Wq[h0], Wq[h1]], axis=1)).astype(BF),
            "wk": np.ascontiguousarray(np.concatenate([Wk[h0], Wk[h1]], axis=1)).astype(BF),
            "wv": np.ascontiguousarray(np.concatenate([Wv[h0], Wv[h1]], axis=1)).astype(BF),
            "bq": np.ascontiguousarray(np.concatenate([bq[h0], bq[h1]]).reshape(DD, 1), np.float32),
            "w0": np.ascontiguousarray(W0[c * DD:(c + 1) * DD, :]).astype(BF),
        })
    return in_maps


def _numpy_reference(x, z, mask, Wq, bq, Wk, bk, Wv, bv, W0, b0):
    # general-mask fallback (not the benchmarked path; harness mask is all-ones)
    x = x.astype(np.float64); z = z.astype(np.float64)
    q = np.einsum("se,hed->hsd", x, Wq) + bq[:, None, :]
    k = np.einsum("te,hed->htd", z, Wk) + bk[:, None, :]
    v = np.einsum("te,hem->htm", z, Wv) + bv[:, None, :]
    s = np.einsum("hsd,htd->hst", q, k) / np.sqrt(np.float64(D))
    s = np.where(mask[None, :, :] == 0, -np.inf, s)
    s = s - s.max(axis=-1, keepdims=True)
    e = np.exp(s)
    a = e / e.sum(axis=-1, keepdims=True)
    o = np.einsum("hst,htm->hsm", a, v)
    o = np.transpose(o, (1, 0, 2)).reshape(S, H * MD)
    return (o @ W0 + b0).astype(np.float32)


def kernel(x, z, mask, Wq, bq, Wk, bk, Wv, bv, W0, b0):
    global LAST_EXEC_TIME_NS, LAST_RESULTS
    arrs = {k: np.asarray(v) for k, v in dict(
        x=x, z=z, mask=mask, Wq=Wq, bq=bq, Wk=Wk, bk=bk, Wv=Wv, bv=bv,
        W0=W0, b0=b0).items()}
    if not bool((arrs["mask"] != 0).all()):
        return _numpy_reference(**arrs)

    from concourse.bass_utils import run_bass_kernel_spmd

    nc = _get_nc()
    in_maps = _make_in_maps(
        arrs["x"], arrs["z"], arrs["Wq"], arrs["bq"], arrs["Wk"],
        arrs["Wv"], arrs["W0"])
    trace = bool(os.environ.get("KERNEL_TRACE"))
    kw = {}
    td = os.environ.get("KERNEL_TRACE_DIR")
    if td:
        os.makedirs(td, exist_ok=True)
        kw["tmpdir"] = td
    res = run_bass_kernel_spmd(
        nc, in_maps, core_ids=list(range(NCORES)), trace=trace, **kw
    )
    LAST_EXEC_TIME_NS = res.exec_time_ns
    LAST_RESULTS = res
    acc = np.zeros((S, O), dtype=np.float32)
    for rm in res.results:
        acc += rm["out"]
    # bv is not applied on-device: sum_t softmax * bv == bv, so it folds
    # into the final bias through W0.
    b0p = (arrs["b0"].astype(np.float64)
           + arrs["bv"].reshape(-1).astype(np.float64) @ arrs["W0"].astype(np.float64))
    acc += b0p.astype(np.float32)[None, :]
    return acc
